# revision 1
# baseline (speedup 1.0000x reference)
"""GRU sequence encoder (DiscSeqRNNEncoder) for 8x TRN2 NeuronCores.

Strategy: pure data-parallel over the batch (1024 rows/core).  On-device
everything lives in "transposed" layout [hidden/gate on partitions, batch on
free] so the recurrent state never needs a transpose.  Host-side prep does
the embedding gather into a transposed fp16 stream with an appended ones-row
(so the PE matmuls fold all biases in), plus the n-gate input projection
table gather.  Per time step the PE accumulates r/z pre-activations
(input + recurrent halves) directly in PSUM, ScalarE applies sigmoids and a
tanh, and DVE/GPSIMD do the remaining elementwise ops (fp16, with a fused
scalar_tensor_tensor for r*(hn+b_hh_n)).  Two half-batch chains (512 each)
pipeline through the engines to hide the serial dependency of the
recurrence.

All constants arrive in ONE packed DMA and each step's inputs (embeddings
stream + n-gate input projections) in ONE DMA, prefetched two steps ahead.
Multi-wait legalization (one sync wait per hardware instruction) is handled
by Bacc.compile()'s generate_event_semaphores pass.
"""

import numpy as np

import concourse.bass as bass
import concourse.tile as tile
from concourse import bacc
from concourse import mybir
from concourse.bass_utils import run_bass_kernel_spmd

F16 = mybir.dt.float16
F32 = mybir.dt.float32

B, L = 8192, 64
NV, E, H, OUT = 1000, 64, 128, 128
N_CORES = 8
BL = B // N_CORES          # batch rows per core
EA = 80                    # embed (64) + ones row (1) padded to 80 partitions
NCONST = 1025              # packed const block free size

_BUILD_CACHE = {}


def build_nc(n_steps=L, bl=BL, n_chains=2, prefetch=4, repeats=1,
             psrz_bufs=3, pshn_bufs=2, gates_bufs=8, h_bufs=2,
             merged_sigma=False, zh_on_dve="alt", zc_alt=False, nzc_alt=False,
             path_prio=None, stt_split=False, split_rz=True, u_alt=False,
             stt_pool_cols=0, zc_dve_all=False, sigma_z_early=False):
    """Build the single-core Bass/Tile program (SPMD across 8 cores).

    repeats > 1 re-runs the whole recurrence (for differential wall-clock
    timing); numerics then chain h across repeats, which is fine for timing.
    """
    bc = bl // n_chains  # batch per chain
    nc = bacc.Bacc("TRN2", target_bir_lowering=False, debug=False)

    st_d = nc.dram_tensor("stream", [n_steps, H, 2 * bl], F16,
                          kind="ExternalInput").ap()
    cb_d = nc.dram_tensor("consts", [H, NCONST], F16, kind="ExternalInput").ap()
    out_d = nc.dram_tensor("out", [bl, OUT], F32, kind="ExternalOutput").ap()

    AF = mybir.ActivationFunctionType
    OP = mybir.AluOpType
    import contextlib

    with tile.TileContext(nc) as tc:
        cpool = tc.alloc_tile_pool(name="consts", bufs=1)
        stpool = tc.alloc_tile_pool(name="stream", bufs=prefetch)
        hpool = tc.alloc_tile_pool(name="hstate", bufs=h_bufs)
        gpool = tc.alloc_tile_pool(name="gates", bufs=gates_bufs)
        psrz = tc.alloc_tile_pool(name="psrz", bufs=psrz_bufs, space="PSUM")
        psz2 = (tc.alloc_tile_pool(name="psz2", bufs=psrz_bufs, space="PSUM")
                if split_rz else None)
        pshn = tc.alloc_tile_pool(name="pshn", bufs=pshn_bufs, space="PSUM")

        cb = cpool.tile([H, NCONST], F16, name="cb_sb")
        nc.sync.dma_start(cb[:], cb_d[:])
        w_r = cb[0:EA, 0:128]
        w_z = cb[0:EA, 128:256]
        whh_r = cb[:, 256:384]
        whh_z = cb[:, 384:512]
        whh_n = cb[:, 512:640]
        woutT = cb[:, 640:768]
        bhn = cb[:, 768:769]
        ones1 = cb[0:1, 769:897]
        bout1 = cb[0:1, 897:1025]

        # initial hidden state = 0
        h = []
        for c in range(n_chains):
            h0 = hpool.tile([H, bc], F16, name=f"h0_{c}", tag=f"h{c}")
            nc.gpsimd.memset(h0[:], 0.0)
            h.append(h0)

        # dummy sigmoid on the zeroed h0 pulls the ACT table load
        # (~2.7us) into the prologue, hidden behind the input DMAs
        warm = gpool.tile([H, 8], F16, name="warm_sb", tag="warm")
        nc.scalar.activation(warm[:], h[0][:, 0:8], AF.Sigmoid)

        # stream prefetch, 2 steps ahead
        streams = {}

        def issue_stream(t):
            st = stpool.tile([H, 2 * bl], F16, name="st_t", tag="st")
            dma = nc.sync.dma_start(st[:], st_d[t % n_steps])
            streams[t] = (st, dma)

        issue_stream(0)
        total_steps = n_steps * repeats
        if total_steps > 1:
            issue_stream(1)

        for t in range(total_steps):
            if t + 2 < total_steps:
                issue_stream(t + 2)
            st, st_dma = streams.pop(t)
            et = st[0:EA, 0:bl]
            xn = st[:, bl:2 * bl]

            for c in range(n_chains):
                ecol = et[:, c * bc:(c + 1) * bc]
                xcol = xn[:, c * bc:(c + 1) * bc]

                if split_rz:
                    r_ps = psrz.tile([H, bc], F32, name="r_ps", tag="rz")
                    z_ps = psz2.tile([H, bc], F32, name="z_ps", tag="z2")
                else:
                    rz_ps = psrz.tile([H, 2 * bc], F32, name="rz_ps", tag="rz")
                    r_ps = rz_ps[:, 0:bc]
                    z_ps = rz_ps[:, bc:2 * bc]
                hn_ps = pshn.tile([H, bc], F32, name="hn_ps", tag="hn")

                # pre-activations: input half first (no dependence on h),
                # then recurrent half accumulates on top
                nc.tensor.matmul(r_ps[:], w_r, ecol,
                                 start=True, stop=False)
                nc.tensor.matmul(z_ps[:], w_z, ecol,
                                 start=True, stop=False)
                nc.tensor.matmul(r_ps[:], whh_r, h[c][:],
                                 start=False, stop=True)
                nc.tensor.matmul(z_ps[:], whh_z, h[c][:],
                                 start=False, stop=True)
                nc.tensor.matmul(hn_ps[:], whh_n, h[c][:],
                                 start=True, stop=True)

                # Critical path per step is h -> hg_r MM -> sigmoid(r) ->
                # tt -> u -> tanh -> nzc -> h_new.  Everything z-related is
                # off-path: sigmoid(z) feeds zc = 1-z and zh = z*h, both on
                # GPSIMD, so only two DVE ops follow the tanh.
                prio = (lambda: tc.high_priority(offset=path_prio)) \
                    if path_prio else contextlib.nullcontext
                rz = gpool.tile([H, 2 * bc], F16, name="rz_sb", tag="rz_sb")
                tt = gpool.tile([H, bc], F16, name="tt_sb", tag="tt")
                u = gpool.tile([H, bc], F16, name="u_sb", tag="u")
                n_sb = gpool.tile([H, bc], F16, name="n_sb", tag="n")
                with prio():
                    if merged_sigma:
                        nc.scalar.activation(rz[:], rz_ps[:], AF.Sigmoid)
                    else:
                        nc.scalar.activation(rz[:, 0:bc], r_ps[:],
                                             AF.Sigmoid)
                        if sigma_z_early:
                            nc.scalar.activation(rz[:, bc:2 * bc], z_ps[:],
                                                 AF.Sigmoid)
                    # tt = (hn + b_hh_n) * r
                    spc = stt_pool_cols
                    dvc = bc - spc
                    if spc > 0:
                        nc.gpsimd.scalar_tensor_tensor(
                            tt[:, dvc:bc], hn_ps[:, dvc:bc], bhn,
                            rz[:, dvc:bc], op0=OP.add, op1=OP.mult)
                    if stt_split and spc == 0:
                        hb = bc // 2
                        nc.vector.scalar_tensor_tensor(
                            tt[:, 0:hb], hn_ps[:, 0:hb], bhn, rz[:, 0:hb],
                            op0=OP.add, op1=OP.mult)
                        nc.vector.scalar_tensor_tensor(
                            tt[:, hb:bc], hn_ps[:, hb:bc], bhn,
                            rz[:, hb:bc], op0=OP.add, op1=OP.mult)
                    elif dvc > 0:
                        nc.vector.scalar_tensor_tensor(
                            tt[:, 0:dvc], hn_ps[:, 0:dvc], bhn, rz[:, 0:dvc],
                            op0=OP.add, op1=OP.mult)
                    if u_alt and c == 1:
                        nc.gpsimd.tensor_add(u[:], tt[:], xcol)
                    else:
                        nc.vector.tensor_add(u[:], tt[:], xcol)
                    nc.scalar.activation(n_sb[:], u[:], AF.Tanh)

                if not merged_sigma and not sigma_z_early:
                    nc.scalar.activation(rz[:, bc:2 * bc], z_ps[:],
                                         AF.Sigmoid)
                zc = gpool.tile([H, bc], F16, name="zc_sb", tag="zc")
                if zc_dve_all or (zc_alt and c == 0):
                    nc.vector.tensor_scalar(zc[:], rz[:, bc:2 * bc], -1.0, 1.0,
                                            OP.mult, OP.add)
                else:
                    nc.gpsimd.tensor_scalar(zc[:], rz[:, bc:2 * bc], -1.0, 1.0,
                                            OP.mult, OP.add)
                zh = gpool.tile([H, bc], F16, name="zh_sb", tag="zh")
                if zh_on_dve is True or (zh_on_dve == "alt" and c == 1) \
                        or (zh_on_dve == "alt0" and c == 0):
                    nc.vector.tensor_mul(zh[:], rz[:, bc:2 * bc], h[c][:])
                else:
                    nc.gpsimd.tensor_mul(zh[:], rz[:, bc:2 * bc], h[c][:])

                # h_new = n*(1-z) + z*h
                nzc = gpool.tile([H, bc], F16, name="nzc_sb", tag="nzc")
                h_new = hpool.tile([H, bc], F16, name=f"hn_{c}", tag=f"h{c}")
                with prio():
                    if nzc_alt and c == 0:
                        nc.gpsimd.tensor_mul(nzc[:], n_sb[:], zc[:])
                    else:
                        nc.vector.tensor_mul(nzc[:], n_sb[:], zc[:])
                    nc.vector.tensor_add(h_new[:], nzc[:], zh[:])
                h[c] = h_new

        # output head: out[b, :] = h_last[:, b] . W_outT + b_out
        for c in range(n_chains):
            for bt in range(bc // H):
                o_ps = pshn.tile([H, OUT], F32, name="o_ps", tag="hn")
                lhs = h[c][:, bt * H:(bt + 1) * H]
                nc.tensor.matmul(o_ps[:], lhs, woutT,
                                 start=True, stop=False)
                nc.tensor.matmul(o_ps[:], ones1, bout1,
                                 start=False, stop=True)
                o_sb = gpool.tile([H, OUT], F32, name="o_sb", tag="osb")
                nc.scalar.activation(o_sb[:], o_ps[:], AF.Copy)
                r0 = c * bc + bt * H
                nc.sync.dma_start(out_d[r0:r0 + H], o_sb[:])

        pools = [pshn] + ([psz2] if split_rz else []) + [psrz, gpool, hpool, stpool, cpool]
        for p in pools:
            p.release()

    nc.compile()
    return nc


def _host_prep(inputs, n_steps=L, bl=BL):
    """Shared (weights) + per-core (streams) host-side layout prep."""
    x = np.asarray(inputs["x"]).astype(np.int64)
    embed = np.asarray(inputs["embed"], dtype=np.float32)
    W_ih = np.asarray(inputs["W_ih"], dtype=np.float32)
    W_hh = np.asarray(inputs["W_hh"], dtype=np.float32)
    b_ih = np.asarray(inputs["b_ih"], dtype=np.float32)
    b_hh = np.asarray(inputs["b_hh"], dtype=np.float32)
    W_out = np.asarray(inputs["W_out"], dtype=np.float32)
    b_out = np.asarray(inputs["b_out"], dtype=np.float32)

    def aug_w(g):
        # lhsT [EA, H]: rows 0:64 = W_ih[g].T, row 64 = combined bias, rest 0
        w = np.zeros((EA, H), np.float16)
        w[:E] = W_ih[g * H:(g + 1) * H].T.astype(np.float16)
        w[E] = (b_ih[g * H:(g + 1) * H] + b_hh[g * H:(g + 1) * H]).astype(np.float16)
        return w

    cb = np.zeros((H, NCONST), np.float16)
    cb[0:EA, 0:128] = aug_w(0)
    cb[0:EA, 128:256] = aug_w(1)
    cb[:, 256:384] = W_hh[0:H].T.astype(np.float16)
    cb[:, 384:512] = W_hh[H:2 * H].T.astype(np.float16)
    cb[:, 512:640] = W_hh[2 * H:3 * H].T.astype(np.float16)
    cb[:, 640:768] = W_out.T.astype(np.float16)
    cb[:, 768] = b_hh[2 * H:3 * H].astype(np.float16)
    cb[0, 769:897] = 1.0
    cb[0, 897:1025] = b_out.astype(np.float16)

    # embed table with ones column for the bias rows of the aug weights
    T_aug = np.zeros((NV, EA), np.float16)
    T_aug[:, :E] = embed.astype(np.float16)
    T_aug[:, E] = 1.0
    # n-gate input projection table (bias folded in)
    G_n = (embed @ W_ih[2 * H:3 * H].T + b_ih[2 * H:3 * H]).astype(np.float16)

    per_core = []
    n_cores = x.shape[0] // bl
    for i in range(n_cores):
        xc = x[i * bl:(i + 1) * bl, :n_steps]              # [bl, n_steps]
        stream = np.zeros((n_steps, H, 2 * bl), np.float16)
        stream[:, 0:EA, 0:bl] = T_aug[xc].transpose(1, 2, 0)
        stream[:, :, bl:2 * bl] = G_n[xc].transpose(1, 2, 0)
        per_core.append({"stream": stream, "consts": cb})
    return per_core


def _run(inputs, trace=False, **kw):
    if "full" not in _BUILD_CACHE:
        _BUILD_CACHE["full"] = build_nc()
    nc = _BUILD_CACHE["full"]
    in_maps = _host_prep(inputs)
    res = run_bass_kernel_spmd(nc, in_maps, list(range(N_CORES)), trace=trace, **kw)
    out = np.concatenate([res.results[i]["out"] for i in range(N_CORES)], axis=0)
    return out.astype(np.float32), res


def kernel(**inputs) -> np.ndarray:
    out, _ = _run(inputs)
    return out



# revision 10
# speedup vs baseline: 1.1107x; 1.1107x over previous
"""GRU sequence encoder (DiscSeqRNNEncoder) for 8x TRN2 NeuronCores.

Strategy: pure data-parallel over the batch (1024 rows/core).  On-device
everything lives in "transposed" layout [hidden/gate on partitions, batch on
free] so the recurrent state never needs a transpose.  Host-side prep does
the embedding gather into a transposed fp16 stream with an appended ones-row
(so the PE matmuls fold all biases in), plus the n-gate input projection
table gather.  Per time step the PE accumulates r/z pre-activations
(input + recurrent halves) directly in PSUM, ScalarE applies sigmoids and a
tanh, and DVE/GPSIMD do the remaining elementwise ops (fp16, with a fused
scalar_tensor_tensor for r*(hn+b_hh_n)).  Two half-batch chains (512 each)
pipeline through the engines to hide the serial dependency of the
recurrence.

All constants arrive in ONE packed DMA and each step's inputs (embeddings
stream + n-gate input projections) in ONE DMA, prefetched two steps ahead.
Multi-wait legalization (one sync wait per hardware instruction) is handled
by Bacc.compile()'s generate_event_semaphores pass.
"""

import numpy as np

import concourse.bass as bass
import concourse.tile as tile
from concourse import bacc
from concourse import mybir
from concourse.bass_utils import run_bass_kernel_spmd

F16 = mybir.dt.float16
F32 = mybir.dt.float32

B, L = 8192, 64
NV, E, H, OUT = 1000, 64, 128, 128
N_CORES = 8
BL = B // N_CORES          # batch rows per core
EA = 80                    # embed (64) + ones row (1) padded to 80 partitions
NCONST = 1025              # packed const block free size
NCONST3 = 2177             # v3 const block (bhn row + ones_bl row)

_BUILD_CACHE = {}


def build_v3(n_steps=L, bl=BL, n_chains=2, prefetch=4, repeats=1,
             gates_bufs=8, h_bufs=2,
             tt_eng="vv", u_eng="vv", d_eng="vv", dl_eng="vv", hup_eng="pp",
             sz_late=False, mm_split=1, prio=None, dl_split=1):
    """Delta-form GRU: persistent PSUM accumulators bank_r/z/n hold the
    running recurrent pre-activations (U·h_t + bias); each step adds
    W·Δe_t (embedding delta, streamed) and U·Δ_{t-1} (Δ = h-increment).
    z-gate weights are negated on host so σ yields z' = 1-z directly:
        n  = tanh(xn + r·bank_n)         bank_n = U_n·h + b_hh_n
        Δ  = z'·(n − h);   h += Δ
    Engine flags: per-op 2-char string, one of 'v' (DVE) / 'p' (GPSIMD)
    per chain.
    """
    bc = bl // n_chains
    nc = bacc.Bacc("TRN2", target_bir_lowering=False, debug=False)

    st_d = nc.dram_tensor("stream", [n_steps, H, 2 * bl], F16,
                          kind="ExternalInput").ap()
    cb_d = nc.dram_tensor("consts", [H, NCONST3], F16, kind="ExternalInput").ap()
    out_d = nc.dram_tensor("out", [bl, OUT], F32, kind="ExternalOutput").ap()

    AF = mybir.ActivationFunctionType
    OP = mybir.AluOpType
    import contextlib

    def eng(flag, c):
        return nc.vector if flag[c] == "v" else nc.gpsimd

    with tile.TileContext(nc) as tc:
        cpool = tc.alloc_tile_pool(name="consts", bufs=1)
        stpool = tc.alloc_tile_pool(name="stream", bufs=prefetch)
        hpool = tc.alloc_tile_pool(name="hstate", bufs=h_bufs)
        gpool = tc.alloc_tile_pool(name="gates", bufs=gates_bufs)
        bpool = tc.alloc_tile_pool(name="banks", bufs=1, space="PSUM")

        cb = cpool.tile([H, NCONST3], F16, name="cb_sb")
        nc.sync.dma_start(cb[:], cb_d[:])
        w_r = cb[0:EA, 0:128]
        w_z = cb[0:EA, 128:256]          # negated on host
        whh_r = cb[:, 256:384]
        whh_z = cb[:, 384:512]           # negated on host
        whh_n = cb[:, 512:640]
        woutT = cb[:, 640:768]
        ones1 = cb[0:1, 769:897]
        bout1 = cb[0:1, 897:1025]
        bhn_row = cb[0:1, 1025:1153]     # b_hh_n as a [1,128] lhsT row
        ones_bl = cb[0:1, 1153:1153 + bl]  # ones row for rank-1 bias fills

        # persistent PSUM accumulators, one bank each
        bank_r, bank_z, bank_n = [], [], []
        for c in range(n_chains):
            bank_r.append(bpool.tile([H, bc], F32, name=f"bkr{c}", tag=f"bkr{c}"))
            bank_z.append(bpool.tile([H, bc], F32, name=f"bkz{c}", tag=f"bkz{c}"))
            bank_n.append(bpool.tile([H, bc], F32, name=f"bkn{c}", tag=f"bkn{c}"))

        h = []
        delta = [None] * n_chains
        for c in range(n_chains):
            h0 = hpool.tile([H, bc], F16, name=f"h0_{c}", tag=f"h{c}")
            nc.gpsimd.memset(h0[:], 0.0)
            h.append(h0)

        # dummy sigmoid pulls the ACT table load into the prologue
        warm = gpool.tile([H, 8], F16, name="warm_sb", tag="warm")
        nc.scalar.activation(warm[:], h[0][:, 0:8], AF.Sigmoid)
        # dummy matmuls burn through the PE p-state ramp while DMAs land
        wps = bpool.tile([H, bc], F32, name="wps", tag="wps")
        for _ in range(6):
            nc.tensor.matmul(wps[:], h[0][:], h[0][:],
                             start=True, stop=True, skip_group_check=True)

        streams = {}

        def issue_stream(t):
            st = stpool.tile([H, 2 * bl], F16, name="st_t", tag="st")
            dma = nc.sync.dma_start(st[:], st_d[t % n_steps])
            streams[t] = (st, dma)

        issue_stream(0)
        total_steps = n_steps * repeats
        if total_steps > 1:
            issue_stream(1)

        def input_mms(t, c, st):
            # W·Δe_t accumulation — depends only on the streamed Δe, so it
            # can run as soon as the banks' previous-step reads are done.
            de = st[0:EA, 0:bl]
            decol = de[:, c * bc:(c + 1) * bc]
            first = t == 0
            nc.tensor.matmul(bank_r[c][:], w_r, decol,
                             start=first, stop=first,
                             skip_group_check=True)
            nc.tensor.matmul(bank_z[c][:], w_z, decol,
                             start=first, stop=first,
                             skip_group_check=True)
            if first:
                # ones row → rank-1 bias fill of bank_n
                nc.tensor.matmul(bank_n[c][:], bhn_row,
                                 ones_bl[:, c * bc:(c + 1) * bc],
                                 start=True, stop=True,
                                 skip_group_check=True)

        for c in range(n_chains):
            input_mms(0, c, streams[0][0])

        for t in range(total_steps):
            if t + 2 < total_steps:
                issue_stream(t + 2)
            st, _ = streams.pop(t)
            xn = st[:, bl:2 * bl]

            for c in range(n_chains):
                xcol = xn[:, c * bc:(c + 1) * bc]

                if t > 0:
                    dl = delta[c]
                    for s in range(mm_split):
                        sl = slice(s * bc // mm_split,
                                   (s + 1) * bc // mm_split)
                        nc.tensor.matmul(bank_r[c][:, sl], whh_r, dl[:, sl],
                                         start=False, stop=True,
                                         skip_group_check=True)
                    nc.tensor.matmul(bank_n[c][:], whh_n, dl[:],
                                     start=False, stop=True,
                                     skip_group_check=True)
                    for s in range(mm_split):
                        sl = slice(s * bc // mm_split,
                                   (s + 1) * bc // mm_split)
                        nc.tensor.matmul(bank_z[c][:, sl], whh_z, dl[:, sl],
                                         start=False, stop=True,
                                         skip_group_check=True)

                pctx = (lambda: tc.high_priority(offset=prio)) \
                    if prio else contextlib.nullcontext
                rz = gpool.tile([H, 2 * bc], F16, name="rz_sb", tag=f"rz{c}")
                tt = gpool.tile([H, bc], F16, name="tt_sb", tag=f"tt{c}")
                u = gpool.tile([H, bc], F16, name="u_sb", tag=f"u{c}")
                n_sb = gpool.tile([H, bc], F16, name="n_sb", tag=f"n{c}")
                with pctx():
                    nc.scalar.activation(rz[:, 0:bc], bank_r[c][:], AF.Sigmoid)
                if not sz_late:
                    nc.scalar.activation(rz[:, bc:2 * bc], bank_z[c][:],
                                         AF.Sigmoid)
                with pctx():
                    eng(tt_eng, c).tensor_mul(tt[:], bank_n[c][:], rz[:, 0:bc])
                    eng(u_eng, c).tensor_add(u[:], tt[:], xcol)
                    nc.scalar.activation(n_sb[:], u[:], AF.Tanh)
                if sz_late:
                    nc.scalar.activation(rz[:, bc:2 * bc], bank_z[c][:],
                                         AF.Sigmoid)

                if t + 1 < total_steps:
                    input_mms(t + 1, c, streams[t + 1][0])

                d = gpool.tile([H, bc], F16, name="d_sb", tag=f"d{c}")
                dl_new = gpool.tile([H, bc], F16, name="dl_sb", tag=f"dl{c}")
                h_new = hpool.tile([H, bc], F16, name=f"hn_{c}", tag=f"h{c}")
                with pctx():
                    eng(d_eng, c).tensor_sub(d[:], n_sb[:], h[c][:])
                    for s in range(dl_split):
                        sl = slice(s * bc // dl_split, (s + 1) * bc // dl_split)
                        eng(dl_eng, c).tensor_mul(dl_new[:, sl],
                                                  rz[:, bc + sl.start:bc + sl.stop],
                                                  d[:, sl])
                eng(hup_eng, c).tensor_add(h_new[:], h[c][:], dl_new[:])
                delta[c] = dl_new
                h[c] = h_new

        # output head: out[b, :] = h_last[:, b] . W_outT + b_out
        # (DMA reads the PSUM result directly; 2 rotating PSUM bufs)
        opool = tc.alloc_tile_pool(name="outps", bufs=2, space="PSUM")
        for c in range(n_chains):
            for bt in range(bc // H):
                o_ps = opool.tile([H, OUT], F32, name="o_ps", tag="ops")
                lhs = h[c][:, bt * H:(bt + 1) * H]
                nc.tensor.matmul(o_ps[:], lhs, woutT,
                                 start=True, stop=False,
                                 skip_group_check=True)
                nc.tensor.matmul(o_ps[:], ones1, bout1,
                                 start=False, stop=True,
                                 skip_group_check=True)
                r0 = c * bc + bt * H
                nc.sync.dma_start(out_d[r0:r0 + H], o_ps[:])

        for p in [opool, bpool, gpool, hpool, stpool, cpool]:
            p.release()

    nc.compile()
    return nc


def _host_prep_v3(inputs, n_steps=L, bl=BL):
    """v3 layout: stream carries Δe (embedding deltas) + xn; z-gate
    weights negated; consts gain a b_hh_n lhsT row."""
    x = np.asarray(inputs["x"]).astype(np.int64)
    embed = np.asarray(inputs["embed"], dtype=np.float32)
    W_ih = np.asarray(inputs["W_ih"], dtype=np.float32)
    W_hh = np.asarray(inputs["W_hh"], dtype=np.float32)
    b_ih = np.asarray(inputs["b_ih"], dtype=np.float32)
    b_hh = np.asarray(inputs["b_hh"], dtype=np.float32)
    W_out = np.asarray(inputs["W_out"], dtype=np.float32)
    b_out = np.asarray(inputs["b_out"], dtype=np.float32)

    def aug_w(g, sign=1.0):
        w = np.zeros((EA, H), np.float16)
        w[:E] = (sign * W_ih[g * H:(g + 1) * H].T).astype(np.float16)
        w[E] = (sign * (b_ih[g * H:(g + 1) * H]
                        + b_hh[g * H:(g + 1) * H])).astype(np.float16)
        return w

    cb = np.zeros((H, NCONST3), np.float16)
    cb[0:EA, 0:128] = aug_w(0)
    cb[0:EA, 128:256] = aug_w(1, sign=-1.0)
    cb[:, 256:384] = W_hh[0:H].T.astype(np.float16)
    cb[:, 384:512] = (-W_hh[H:2 * H].T).astype(np.float16)
    cb[:, 512:640] = W_hh[2 * H:3 * H].T.astype(np.float16)
    cb[:, 640:768] = W_out.T.astype(np.float16)
    cb[0, 769:897] = 1.0
    cb[0, 897:1025] = b_out.astype(np.float16)
    cb[0, 1025:1153] = b_hh[2 * H:3 * H].astype(np.float16)
    cb[0, 1153:1153 + bl] = 1.0

    T_aug = np.zeros((NV, EA), np.float32)
    T_aug[:, :E] = embed
    T_aug[:, E] = 1.0
    G_n = (embed @ W_ih[2 * H:3 * H].T + b_ih[2 * H:3 * H]).astype(np.float16)

    per_core = []
    n_cores = x.shape[0] // bl
    for i in range(n_cores):
        xc = x[i * bl:(i + 1) * bl, :n_steps]              # [bl, n_steps]
        e_seq = T_aug[xc]                                  # [bl, T, EA] f32
        de_seq = e_seq.copy()
        de_seq[:, 1:] -= e_seq[:, :-1]
        stream = np.zeros((n_steps, H, 2 * bl), np.float16)
        stream[:, 0:EA, 0:bl] = de_seq.astype(np.float16).transpose(1, 2, 0)
        stream[:, :, bl:2 * bl] = G_n[xc].transpose(1, 2, 0)
        per_core.append({"stream": stream, "consts": cb})
    return per_core


def build_v1(n_steps=L, bl=BL, n_chains=2, prefetch=4, repeats=1,
             psrz_bufs=3, pshn_bufs=2, gates_bufs=8, h_bufs=2,
             merged_sigma=False, zh_on_dve="alt", zc_alt=False, nzc_alt=False,
             path_prio=None, stt_split=False, split_rz=True, u_alt=False,
             stt_pool_cols=0, zc_dve_all=False, sigma_z_early=False):
    """Build the single-core Bass/Tile program (SPMD across 8 cores).

    repeats > 1 re-runs the whole recurrence (for differential wall-clock
    timing); numerics then chain h across repeats, which is fine for timing.
    """
    bc = bl // n_chains  # batch per chain
    nc = bacc.Bacc("TRN2", target_bir_lowering=False, debug=False)

    st_d = nc.dram_tensor("stream", [n_steps, H, 2 * bl], F16,
                          kind="ExternalInput").ap()
    cb_d = nc.dram_tensor("consts", [H, NCONST], F16, kind="ExternalInput").ap()
    out_d = nc.dram_tensor("out", [bl, OUT], F32, kind="ExternalOutput").ap()

    AF = mybir.ActivationFunctionType
    OP = mybir.AluOpType
    import contextlib

    with tile.TileContext(nc) as tc:
        cpool = tc.alloc_tile_pool(name="consts", bufs=1)
        stpool = tc.alloc_tile_pool(name="stream", bufs=prefetch)
        hpool = tc.alloc_tile_pool(name="hstate", bufs=h_bufs)
        gpool = tc.alloc_tile_pool(name="gates", bufs=gates_bufs)
        psrz = tc.alloc_tile_pool(name="psrz", bufs=psrz_bufs, space="PSUM")
        psz2 = (tc.alloc_tile_pool(name="psz2", bufs=psrz_bufs, space="PSUM")
                if split_rz else None)
        pshn = tc.alloc_tile_pool(name="pshn", bufs=pshn_bufs, space="PSUM")

        cb = cpool.tile([H, NCONST], F16, name="cb_sb")
        nc.sync.dma_start(cb[:], cb_d[:])
        w_r = cb[0:EA, 0:128]
        w_z = cb[0:EA, 128:256]
        whh_r = cb[:, 256:384]
        whh_z = cb[:, 384:512]
        whh_n = cb[:, 512:640]
        woutT = cb[:, 640:768]
        bhn = cb[:, 768:769]
        ones1 = cb[0:1, 769:897]
        bout1 = cb[0:1, 897:1025]

        # initial hidden state = 0
        h = []
        for c in range(n_chains):
            h0 = hpool.tile([H, bc], F16, name=f"h0_{c}", tag=f"h{c}")
            nc.gpsimd.memset(h0[:], 0.0)
            h.append(h0)

        # dummy sigmoid on the zeroed h0 pulls the ACT table load
        # (~2.7us) into the prologue, hidden behind the input DMAs
        warm = gpool.tile([H, 8], F16, name="warm_sb", tag="warm")
        nc.scalar.activation(warm[:], h[0][:, 0:8], AF.Sigmoid)

        # stream prefetch, 2 steps ahead
        streams = {}

        def issue_stream(t):
            st = stpool.tile([H, 2 * bl], F16, name="st_t", tag="st")
            dma = nc.sync.dma_start(st[:], st_d[t % n_steps])
            streams[t] = (st, dma)

        issue_stream(0)
        total_steps = n_steps * repeats
        if total_steps > 1:
            issue_stream(1)

        for t in range(total_steps):
            if t + 2 < total_steps:
                issue_stream(t + 2)
            st, st_dma = streams.pop(t)
            et = st[0:EA, 0:bl]
            xn = st[:, bl:2 * bl]

            for c in range(n_chains):
                ecol = et[:, c * bc:(c + 1) * bc]
                xcol = xn[:, c * bc:(c + 1) * bc]

                if split_rz:
                    r_ps = psrz.tile([H, bc], F32, name="r_ps", tag="rz")
                    z_ps = psz2.tile([H, bc], F32, name="z_ps", tag="z2")
                else:
                    rz_ps = psrz.tile([H, 2 * bc], F32, name="rz_ps", tag="rz")
                    r_ps = rz_ps[:, 0:bc]
                    z_ps = rz_ps[:, bc:2 * bc]
                hn_ps = pshn.tile([H, bc], F32, name="hn_ps", tag="hn")

                # pre-activations: input half first (no dependence on h),
                # then recurrent half accumulates on top
                nc.tensor.matmul(r_ps[:], w_r, ecol,
                                 start=True, stop=False)
                nc.tensor.matmul(z_ps[:], w_z, ecol,
                                 start=True, stop=False)
                nc.tensor.matmul(r_ps[:], whh_r, h[c][:],
                                 start=False, stop=True)
                nc.tensor.matmul(z_ps[:], whh_z, h[c][:],
                                 start=False, stop=True)
                nc.tensor.matmul(hn_ps[:], whh_n, h[c][:],
                                 start=True, stop=True)

                # Critical path per step is h -> hg_r MM -> sigmoid(r) ->
                # tt -> u -> tanh -> nzc -> h_new.  Everything z-related is
                # off-path: sigmoid(z) feeds zc = 1-z and zh = z*h, both on
                # GPSIMD, so only two DVE ops follow the tanh.
                prio = (lambda: tc.high_priority(offset=path_prio)) \
                    if path_prio else contextlib.nullcontext
                rz = gpool.tile([H, 2 * bc], F16, name="rz_sb", tag="rz_sb")
                tt = gpool.tile([H, bc], F16, name="tt_sb", tag="tt")
                u = gpool.tile([H, bc], F16, name="u_sb", tag="u")
                n_sb = gpool.tile([H, bc], F16, name="n_sb", tag="n")
                with prio():
                    if merged_sigma:
                        nc.scalar.activation(rz[:], rz_ps[:], AF.Sigmoid)
                    else:
                        nc.scalar.activation(rz[:, 0:bc], r_ps[:],
                                             AF.Sigmoid)
                        if sigma_z_early:
                            nc.scalar.activation(rz[:, bc:2 * bc], z_ps[:],
                                                 AF.Sigmoid)
                    # tt = (hn + b_hh_n) * r
                    spc = stt_pool_cols
                    dvc = bc - spc
                    if spc > 0:
                        nc.gpsimd.scalar_tensor_tensor(
                            tt[:, dvc:bc], hn_ps[:, dvc:bc], bhn,
                            rz[:, dvc:bc], op0=OP.add, op1=OP.mult)
                    if stt_split and spc == 0:
                        hb = bc // 2
                        nc.vector.scalar_tensor_tensor(
                            tt[:, 0:hb], hn_ps[:, 0:hb], bhn, rz[:, 0:hb],
                            op0=OP.add, op1=OP.mult)
                        nc.vector.scalar_tensor_tensor(
                            tt[:, hb:bc], hn_ps[:, hb:bc], bhn,
                            rz[:, hb:bc], op0=OP.add, op1=OP.mult)
                    elif dvc > 0:
                        nc.vector.scalar_tensor_tensor(
                            tt[:, 0:dvc], hn_ps[:, 0:dvc], bhn, rz[:, 0:dvc],
                            op0=OP.add, op1=OP.mult)
                    if u_alt and c == 1:
                        nc.gpsimd.tensor_add(u[:], tt[:], xcol)
                    else:
                        nc.vector.tensor_add(u[:], tt[:], xcol)
                    nc.scalar.activation(n_sb[:], u[:], AF.Tanh)

                if not merged_sigma and not sigma_z_early:
                    nc.scalar.activation(rz[:, bc:2 * bc], z_ps[:],
                                         AF.Sigmoid)
                zc = gpool.tile([H, bc], F16, name="zc_sb", tag="zc")
                if zc_dve_all or (zc_alt and c == 0):
                    nc.vector.tensor_scalar(zc[:], rz[:, bc:2 * bc], -1.0, 1.0,
                                            OP.mult, OP.add)
                else:
                    nc.gpsimd.tensor_scalar(zc[:], rz[:, bc:2 * bc], -1.0, 1.0,
                                            OP.mult, OP.add)
                zh = gpool.tile([H, bc], F16, name="zh_sb", tag="zh")
                if zh_on_dve is True or (zh_on_dve == "alt" and c == 1) \
                        or (zh_on_dve == "alt0" and c == 0):
                    nc.vector.tensor_mul(zh[:], rz[:, bc:2 * bc], h[c][:])
                else:
                    nc.gpsimd.tensor_mul(zh[:], rz[:, bc:2 * bc], h[c][:])

                # h_new = n*(1-z) + z*h
                nzc = gpool.tile([H, bc], F16, name="nzc_sb", tag="nzc")
                h_new = hpool.tile([H, bc], F16, name=f"hn_{c}", tag=f"h{c}")
                with prio():
                    if nzc_alt and c == 0:
                        nc.gpsimd.tensor_mul(nzc[:], n_sb[:], zc[:])
                    else:
                        nc.vector.tensor_mul(nzc[:], n_sb[:], zc[:])
                    nc.vector.tensor_add(h_new[:], nzc[:], zh[:])
                h[c] = h_new

        # output head: out[b, :] = h_last[:, b] . W_outT + b_out
        for c in range(n_chains):
            for bt in range(bc // H):
                o_ps = pshn.tile([H, OUT], F32, name="o_ps", tag="hn")
                lhs = h[c][:, bt * H:(bt + 1) * H]
                nc.tensor.matmul(o_ps[:], lhs, woutT,
                                 start=True, stop=False)
                nc.tensor.matmul(o_ps[:], ones1, bout1,
                                 start=False, stop=True)
                o_sb = gpool.tile([H, OUT], F32, name="o_sb", tag="osb")
                nc.scalar.activation(o_sb[:], o_ps[:], AF.Copy)
                r0 = c * bc + bt * H
                nc.sync.dma_start(out_d[r0:r0 + H], o_sb[:])

        pools = [pshn] + ([psz2] if split_rz else []) + [psrz, gpool, hpool, stpool, cpool]
        for p in pools:
            p.release()

    nc.compile()
    return nc


def _host_prep_v1(inputs, n_steps=L, bl=BL):
    """Shared (weights) + per-core (streams) host-side layout prep."""
    x = np.asarray(inputs["x"]).astype(np.int64)
    embed = np.asarray(inputs["embed"], dtype=np.float32)
    W_ih = np.asarray(inputs["W_ih"], dtype=np.float32)
    W_hh = np.asarray(inputs["W_hh"], dtype=np.float32)
    b_ih = np.asarray(inputs["b_ih"], dtype=np.float32)
    b_hh = np.asarray(inputs["b_hh"], dtype=np.float32)
    W_out = np.asarray(inputs["W_out"], dtype=np.float32)
    b_out = np.asarray(inputs["b_out"], dtype=np.float32)

    def aug_w(g):
        # lhsT [EA, H]: rows 0:64 = W_ih[g].T, row 64 = combined bias, rest 0
        w = np.zeros((EA, H), np.float16)
        w[:E] = W_ih[g * H:(g + 1) * H].T.astype(np.float16)
        w[E] = (b_ih[g * H:(g + 1) * H] + b_hh[g * H:(g + 1) * H]).astype(np.float16)
        return w

    cb = np.zeros((H, NCONST), np.float16)
    cb[0:EA, 0:128] = aug_w(0)
    cb[0:EA, 128:256] = aug_w(1)
    cb[:, 256:384] = W_hh[0:H].T.astype(np.float16)
    cb[:, 384:512] = W_hh[H:2 * H].T.astype(np.float16)
    cb[:, 512:640] = W_hh[2 * H:3 * H].T.astype(np.float16)
    cb[:, 640:768] = W_out.T.astype(np.float16)
    cb[:, 768] = b_hh[2 * H:3 * H].astype(np.float16)
    cb[0, 769:897] = 1.0
    cb[0, 897:1025] = b_out.astype(np.float16)

    # embed table with ones column for the bias rows of the aug weights
    T_aug = np.zeros((NV, EA), np.float16)
    T_aug[:, :E] = embed.astype(np.float16)
    T_aug[:, E] = 1.0
    # n-gate input projection table (bias folded in)
    G_n = (embed @ W_ih[2 * H:3 * H].T + b_ih[2 * H:3 * H]).astype(np.float16)

    per_core = []
    n_cores = x.shape[0] // bl
    for i in range(n_cores):
        xc = x[i * bl:(i + 1) * bl, :n_steps]              # [bl, n_steps]
        stream = np.zeros((n_steps, H, 2 * bl), np.float16)
        stream[:, 0:EA, 0:bl] = T_aug[xc].transpose(1, 2, 0)
        stream[:, :, bl:2 * bl] = G_n[xc].transpose(1, 2, 0)
        per_core.append({"stream": stream, "consts": cb})
    return per_core


import os

VARIANT = os.environ.get("BASS_VARIANT", "v3")


def build_nc(**kw):
    return build_v1(**kw) if VARIANT == "v1" else build_v3(**kw)


def _host_prep(inputs, n_steps=L, bl=BL):
    if VARIANT == "v1":
        return _host_prep_v1(inputs, n_steps=n_steps, bl=bl)
    return _host_prep_v3(inputs, n_steps=n_steps, bl=bl)


def _run(inputs, trace=False, **kw):
    key = "full_" + VARIANT
    if key not in _BUILD_CACHE:
        _BUILD_CACHE[key] = build_nc()
    nc = _BUILD_CACHE[key]
    in_maps = _host_prep(inputs)
    res = run_bass_kernel_spmd(nc, in_maps, list(range(N_CORES)), trace=trace, **kw)
    out = np.concatenate([res.results[i]["out"] for i in range(N_CORES)], axis=0)
    return out.astype(np.float32), res


def kernel(**inputs) -> np.ndarray:
    out, _ = _run(inputs)
    return out



# revision 25
# speedup vs baseline: 1.2887x; 1.1603x over previous
"""GRU sequence encoder (DiscSeqRNNEncoder) for 8x TRN2 NeuronCores.

Strategy: pure data-parallel over the batch (1024 rows/core).  On-device
everything lives in "transposed" layout [hidden/gate on partitions, batch on
free] so the recurrent state never needs a transpose.  Host-side prep does
the embedding gather into a transposed fp16 stream with an appended ones-row
(so the PE matmuls fold all biases in), plus the n-gate input projection
table gather.  Per time step the PE accumulates r/z pre-activations
(input + recurrent halves) directly in PSUM, ScalarE applies sigmoids and a
tanh, and DVE/GPSIMD do the remaining elementwise ops (fp16, with a fused
scalar_tensor_tensor for r*(hn+b_hh_n)).  Two half-batch chains (512 each)
pipeline through the engines to hide the serial dependency of the
recurrence.

All constants arrive in ONE packed DMA and each step's inputs (embeddings
stream + n-gate input projections) in ONE DMA, prefetched two steps ahead.
Multi-wait legalization (one sync wait per hardware instruction) is handled
by Bacc.compile()'s generate_event_semaphores pass.
"""

import numpy as np

import concourse.bass as bass
import concourse.tile as tile
from concourse import bacc
from concourse import mybir
from concourse.bass_utils import run_bass_kernel_spmd

F16 = mybir.dt.float16
F32 = mybir.dt.float32

B, L = 8192, 64
NV, E, H, OUT = 1000, 64, 128, 128
N_CORES = 8
BL = B // N_CORES          # batch rows per core
EA = 80                    # embed (64) + ones row (1) padded to 80 partitions
NCONST = 1025              # packed const block free size
NCONST3 = 2561             # v3 consts (bhn row, ones_bl row, negated U blocks)

_BUILD_CACHE = {}


def build_v3(n_steps=L, bl=BL, n_chains=2, prefetch=4, repeats=1,
             gates_bufs=12, h_bufs=3,
             tt_eng="vv", u_eng="vv", d_eng="vv", dl_eng="vv", hup_eng="vp",
             sz_late=False, mm_split=1, prio=None, dl_split=1, sz_dep=False,
             sz_split=1, zh_split=False, zh_eng="vv", hm_eng="pp",
             hnew_eng="vp"):
    """Delta-form GRU: persistent PSUM accumulators bank_r/z/n hold the
    running recurrent pre-activations (U·h_t + bias); each step adds
    W·Δe_t (embedding delta, streamed) and U·Δ_{t-1} (Δ = h-increment).
    z-gate weights are negated on host so σ yields z' = 1-z directly:
        n  = tanh(xn + r·bank_n)         bank_n = U_n·h + b_hh_n
        Δ  = z'·(n − h);   h += Δ
    Engine flags: per-op 2-char string, one of 'v' (DVE) / 'p' (GPSIMD)
    per chain.
    """
    bc = bl // n_chains
    nc = bacc.Bacc("TRN2", target_bir_lowering=False, debug=False)

    st_d = nc.dram_tensor("stream", [n_steps, H, 2 * bl], F16,
                          kind="ExternalInput").ap()
    cb_d = nc.dram_tensor("consts", [H, NCONST3], F16, kind="ExternalInput").ap()
    out_d = nc.dram_tensor("out", [bl, OUT], F32, kind="ExternalOutput").ap()

    AF = mybir.ActivationFunctionType
    OP = mybir.AluOpType
    import contextlib

    def eng(flag, c):
        return nc.vector if flag[c] == "v" else nc.gpsimd

    with tile.TileContext(nc) as tc:
        cpool = tc.alloc_tile_pool(name="consts", bufs=1)
        stpool = tc.alloc_tile_pool(name="stream", bufs=prefetch)
        hpool = tc.alloc_tile_pool(name="hstate", bufs=h_bufs)
        gpool = tc.alloc_tile_pool(name="gates", bufs=gates_bufs)
        bpool = tc.alloc_tile_pool(name="banks", bufs=1, space="PSUM")

        cb = cpool.tile([H, NCONST3], F16, name="cb_sb")
        nc.sync.dma_start(cb[:], cb_d[:])
        w_r = cb[0:EA, 0:128]
        w_z = cb[0:EA, 128:256]          # negated on host
        whh_r = cb[:, 256:384]
        whh_z = cb[:, 384:512]           # negated on host
        whh_n = cb[:, 512:640]
        woutT = cb[:, 640:768]
        ones1 = cb[0:1, 769:897]
        bout1 = cb[0:1, 897:1025]
        bhn_row = cb[0:1, 1025:1153]     # b_hh_n as a [1,128] lhsT row
        ones_bl = cb[0:1, 1153:1153 + bl]  # ones row for rank-1 bias fills
        whh_rm = cb[:, 2177:2305]        # -W_hh_r.T (zh_split)
        whh_zm = cb[:, 2305:2433]        # +W_hh_z.T (= negated z'-weights)
        whh_nm = cb[:, 2433:2561]        # -W_hh_n.T

        # persistent PSUM accumulators, one bank each
        bank_r, bank_z, bank_n = [], [], []
        for c in range(n_chains):
            bank_r.append(bpool.tile([H, bc], F32, name=f"bkr{c}", tag=f"bkr{c}"))
            bank_z.append(bpool.tile([H, bc], F32, name=f"bkz{c}", tag=f"bkz{c}"))
            bank_n.append(bpool.tile([H, bc], F32, name=f"bkn{c}", tag=f"bkn{c}"))

        h = []
        delta = [None] * n_chains
        zh_tiles = [None] * n_chains
        for c in range(n_chains):
            h0 = hpool.tile([H, bc], F16, name=f"h0_{c}", tag=f"h{c}")
            nc.gpsimd.memset(h0[:], 0.0)
            h.append(h0)

        # dummy sigmoid pulls the ACT table load into the prologue
        warm = gpool.tile([H, 8], F16, name="warm_sb", tag="warm")
        nc.scalar.activation(warm[:], h[0][:, 0:8], AF.Sigmoid)
        # dummy matmuls burn through the PE p-state ramp while DMAs land
        # (they write into bank_r[0], which the first real MM resets with
        # start=True, so the garbage is harmless)
        for _ in range(8):
            nc.tensor.matmul(bank_r[0][:], h[0][:, 0:H], h[0][:],
                             start=True, stop=True, skip_group_check=True)

        streams = {}

        def issue_stream(t):
            st = stpool.tile([H, 2 * bl], F16, name="st_t", tag="st")
            dma = nc.sync.dma_start(st[:], st_d[t % n_steps])
            streams[t] = (st, dma)

        issue_stream(0)
        total_steps = n_steps * repeats
        if total_steps > 1:
            issue_stream(1)

        def input_mms(t, c, st):
            # W·Δe_t accumulation — depends only on the streamed Δe, so it
            # can run as soon as the banks' previous-step reads are done.
            de = st[0:EA, 0:bl]
            decol = de[:, c * bc:(c + 1) * bc]
            first = t == 0
            nc.tensor.matmul(bank_r[c][:], w_r, decol,
                             start=first, stop=first,
                             skip_group_check=True)
            nc.tensor.matmul(bank_z[c][:], w_z, decol,
                             start=first, stop=first,
                             skip_group_check=True)
            if first:
                # ones row → rank-1 bias fill of bank_n
                nc.tensor.matmul(bank_n[c][:], bhn_row,
                                 ones_bl[:, c * bc:(c + 1) * bc],
                                 start=True, stop=True,
                                 skip_group_check=True)

        for c in range(n_chains):
            input_mms(0, c, streams[0][0])

        for t in range(total_steps):
            if t + 2 < total_steps:
                issue_stream(t + 2)
            st, _ = streams.pop(t)
            xn = st[:, bl:2 * bl]

            for c in range(n_chains):
                xcol = xn[:, c * bc:(c + 1) * bc]

                if t > 0:
                    dl = delta[c]
                    for s in range(mm_split):
                        sl = slice(s * bc // mm_split,
                                   (s + 1) * bc // mm_split)
                        nc.tensor.matmul(bank_r[c][:, sl], whh_r, dl[:, sl],
                                         start=False, stop=True,
                                         skip_group_check=True)
                    nc.tensor.matmul(bank_n[c][:], whh_n, dl[:],
                                     start=False, stop=True,
                                     skip_group_check=True)
                    for s in range(mm_split):
                        sl = slice(s * bc // mm_split,
                                   (s + 1) * bc // mm_split)
                        nc.tensor.matmul(bank_z[c][:, sl], whh_z, dl[:, sl],
                                         start=False, stop=True,
                                         skip_group_check=True)
                    if zh_split:
                        # subtractive half: U_neg · zh_{t-1} (zh was ready
                        # mid-previous-step; these run off the critical path)
                        zh_prev = zh_tiles[c]
                        nc.tensor.matmul(bank_r[c][:], whh_rm, zh_prev[:],
                                         start=False, stop=True,
                                         skip_group_check=True)
                        nc.tensor.matmul(bank_n[c][:], whh_nm, zh_prev[:],
                                         start=False, stop=True,
                                         skip_group_check=True)
                        nc.tensor.matmul(bank_z[c][:], whh_zm, zh_prev[:],
                                         start=False, stop=True,
                                         skip_group_check=True)

                pctx = (lambda: tc.high_priority(offset=prio)) \
                    if prio else contextlib.nullcontext
                rz = gpool.tile([H, 2 * bc], F16, name="rz_sb", tag=f"rz{c}")
                tt = gpool.tile([H, bc], F16, name="tt_sb", tag=f"tt{c}")
                u = gpool.tile([H, bc], F16, name="u_sb", tag=f"u{c}")
                n_sb = gpool.tile([H, bc], F16, name="n_sb", tag=f"n{c}")
                with pctx():
                    nc.scalar.activation(rz[:, 0:bc], bank_r[c][:], AF.Sigmoid)
                if not sz_late and not sz_dep:
                    for s in range(sz_split):
                        sl = slice(s * bc // sz_split, (s + 1) * bc // sz_split)
                        nc.scalar.activation(rz[:, bc + sl.start:bc + sl.stop],
                                             bank_z[c][:, sl], AF.Sigmoid)
                with pctx():
                    eng(tt_eng, c).tensor_mul(tt[:], bank_n[c][:], rz[:, 0:bc])
                    eng(u_eng, c).tensor_add(u[:], tt[:], xcol)
                    nc.scalar.activation(n_sb[:], u[:], AF.Tanh)
                if sz_dep:
                    # zero bias derived from the tanh output: forces the
                    # in-order ACT queue to schedule this sigma_z strictly
                    # after the same chain's tanh (it would otherwise
                    # greedily run first and head-of-line block the tanh)
                    bz = gpool.tile([H, 1], F32, name="bz_sb", tag=f"bz{c}")
                    nc.vector.tensor_scalar(bz[:], n_sb[:, 0:1], 0.0, 0.0,
                                            OP.mult, OP.mult)
                    nc.scalar.activation(rz[:, bc:2 * bc], bank_z[c][:],
                                         AF.Sigmoid, bias=bz[:])
                elif sz_late:
                    nc.scalar.activation(rz[:, bc:2 * bc], bank_z[c][:],
                                         AF.Sigmoid)

                if t + 1 < total_steps:
                    input_mms(t + 1, c, streams[t + 1][0])

                h_new = hpool.tile([H, bc], F16, name=f"hn_{c}", tag=f"h{c}")
                if zh_split:
                    # zh = z'*h (early, off-path); hm = h - zh (off-path);
                    # zn = z'*n (path -> feeds next step's U MMs); h' = hm+zn
                    zh_t = gpool.tile([H, bc], F16, name="zh_sb", tag=f"zh{c}")
                    hm = gpool.tile([H, bc], F16, name="hm_sb", tag=f"hm{c}")
                    zn = gpool.tile([H, bc], F16, name="zn_sb", tag=f"zn{c}")
                    eng(zh_eng, c).tensor_mul(zh_t[:], rz[:, bc:2 * bc],
                                              h[c][:])
                    eng(hm_eng, c).tensor_sub(hm[:], h[c][:], zh_t[:])
                    with pctx():
                        eng(d_eng, c).tensor_mul(zn[:], rz[:, bc:2 * bc],
                                                 n_sb[:])
                    eng(hnew_eng, c).tensor_add(h_new[:], hm[:], zn[:])
                    zh_tiles[c] = zh_t
                    delta[c] = zn
                else:
                    d = gpool.tile([H, bc], F16, name="d_sb", tag=f"d{c}")
                    dl_new = gpool.tile([H, bc], F16, name="dl_sb", tag=f"dl{c}")
                    with pctx():
                        eng(d_eng, c).tensor_sub(d[:], n_sb[:], h[c][:])
                        for s in range(dl_split):
                            sl = slice(s * bc // dl_split,
                                       (s + 1) * bc // dl_split)
                            eng(dl_eng, c).tensor_mul(
                                dl_new[:, sl],
                                rz[:, bc + sl.start:bc + sl.stop],
                                d[:, sl])
                    eng(hup_eng, c).tensor_add(h_new[:], h[c][:], dl_new[:])
                    delta[c] = dl_new
                h[c] = h_new

        # output head: out[b, :] = h_last[:, b] . W_outT + b_out
        # PSUM result -> one packed SBUF tile -> ONE output DMA
        opool = tc.alloc_tile_pool(name="outps", bufs=2, space="PSUM")
        n_blk = bl // H
        oapool = tc.alloc_tile_pool(name="oall", bufs=1)
        o_all = oapool.tile([H, n_blk * OUT], F32, name="o_all", tag="oall")
        for c in range(n_chains):
            for bt in range(bc // H):
                o_ps = opool.tile([H, OUT], F32, name="o_ps", tag="ops")
                lhs = h[c][:, bt * H:(bt + 1) * H]
                nc.tensor.matmul(o_ps[:], lhs, woutT,
                                 start=True, stop=False,
                                 skip_group_check=True)
                nc.tensor.matmul(o_ps[:], ones1, bout1,
                                 start=False, stop=True,
                                 skip_group_check=True)
                blk = c * (bc // H) + bt
                nc.vector.tensor_copy(o_all[:, blk * OUT:(blk + 1) * OUT],
                                      o_ps[:])
        nc.sync.dma_start(
            out_d.rearrange("(blk p) o -> p blk o", p=H), o_all[:])

        for p in [oapool, opool, bpool, gpool, hpool, stpool, cpool]:
            p.release()

    nc.compile()
    return nc


def _host_prep_v3(inputs, n_steps=L, bl=BL):
    """v3 layout: stream carries Δe (embedding deltas) + xn; z-gate
    weights negated; consts gain a b_hh_n lhsT row."""
    x = np.asarray(inputs["x"]).astype(np.int64)
    embed = np.asarray(inputs["embed"], dtype=np.float32)
    W_ih = np.asarray(inputs["W_ih"], dtype=np.float32)
    W_hh = np.asarray(inputs["W_hh"], dtype=np.float32)
    b_ih = np.asarray(inputs["b_ih"], dtype=np.float32)
    b_hh = np.asarray(inputs["b_hh"], dtype=np.float32)
    W_out = np.asarray(inputs["W_out"], dtype=np.float32)
    b_out = np.asarray(inputs["b_out"], dtype=np.float32)

    def aug_w(g, sign=1.0):
        w = np.zeros((EA, H), np.float16)
        w[:E] = (sign * W_ih[g * H:(g + 1) * H].T).astype(np.float16)
        w[E] = (sign * (b_ih[g * H:(g + 1) * H]
                        + b_hh[g * H:(g + 1) * H])).astype(np.float16)
        return w

    cb = np.zeros((H, NCONST3), np.float16)
    cb[0:EA, 0:128] = aug_w(0)
    cb[0:EA, 128:256] = aug_w(1, sign=-1.0)
    cb[:, 256:384] = W_hh[0:H].T.astype(np.float16)
    cb[:, 384:512] = (-W_hh[H:2 * H].T).astype(np.float16)
    cb[:, 512:640] = W_hh[2 * H:3 * H].T.astype(np.float16)
    cb[:, 640:768] = W_out.T.astype(np.float16)
    cb[0, 769:897] = 1.0
    cb[0, 897:1025] = b_out.astype(np.float16)
    cb[0, 1025:1153] = b_hh[2 * H:3 * H].astype(np.float16)
    cb[0, 1153:1153 + bl] = 1.0
    cb[:, 2177:2305] = (-W_hh[0:H].T).astype(np.float16)
    cb[:, 2305:2433] = W_hh[H:2 * H].T.astype(np.float16)
    cb[:, 2433:2561] = (-W_hh[2 * H:3 * H].T).astype(np.float16)

    T_aug = np.zeros((NV, EA), np.float32)
    T_aug[:, :E] = embed
    T_aug[:, E] = 1.0
    G_n = (embed @ W_ih[2 * H:3 * H].T + b_ih[2 * H:3 * H]).astype(np.float16)

    per_core = []
    n_cores = x.shape[0] // bl
    for i in range(n_cores):
        xc = x[i * bl:(i + 1) * bl, :n_steps]              # [bl, n_steps]
        e_seq = T_aug[xc]                                  # [bl, T, EA] f32
        de_seq = e_seq.copy()
        de_seq[:, 1:] -= e_seq[:, :-1]
        stream = np.zeros((n_steps, H, 2 * bl), np.float16)
        stream[:, 0:EA, 0:bl] = de_seq.astype(np.float16).transpose(1, 2, 0)
        stream[:, :, bl:2 * bl] = G_n[xc].transpose(1, 2, 0)
        per_core.append({"stream": stream, "consts": cb})
    return per_core


def build_v1(n_steps=L, bl=BL, n_chains=2, prefetch=4, repeats=1,
             psrz_bufs=3, pshn_bufs=2, gates_bufs=8, h_bufs=2,
             merged_sigma=False, zh_on_dve="alt", zc_alt=False, nzc_alt=False,
             path_prio=None, stt_split=False, split_rz=True, u_alt=False,
             stt_pool_cols=0, zc_dve_all=False, sigma_z_early=False):
    """Build the single-core Bass/Tile program (SPMD across 8 cores).

    repeats > 1 re-runs the whole recurrence (for differential wall-clock
    timing); numerics then chain h across repeats, which is fine for timing.
    """
    bc = bl // n_chains  # batch per chain
    nc = bacc.Bacc("TRN2", target_bir_lowering=False, debug=False)

    st_d = nc.dram_tensor("stream", [n_steps, H, 2 * bl], F16,
                          kind="ExternalInput").ap()
    cb_d = nc.dram_tensor("consts", [H, NCONST], F16, kind="ExternalInput").ap()
    out_d = nc.dram_tensor("out", [bl, OUT], F32, kind="ExternalOutput").ap()

    AF = mybir.ActivationFunctionType
    OP = mybir.AluOpType
    import contextlib

    with tile.TileContext(nc) as tc:
        cpool = tc.alloc_tile_pool(name="consts", bufs=1)
        stpool = tc.alloc_tile_pool(name="stream", bufs=prefetch)
        hpool = tc.alloc_tile_pool(name="hstate", bufs=h_bufs)
        gpool = tc.alloc_tile_pool(name="gates", bufs=gates_bufs)
        psrz = tc.alloc_tile_pool(name="psrz", bufs=psrz_bufs, space="PSUM")
        psz2 = (tc.alloc_tile_pool(name="psz2", bufs=psrz_bufs, space="PSUM")
                if split_rz else None)
        pshn = tc.alloc_tile_pool(name="pshn", bufs=pshn_bufs, space="PSUM")

        cb = cpool.tile([H, NCONST], F16, name="cb_sb")
        nc.sync.dma_start(cb[:], cb_d[:])
        w_r = cb[0:EA, 0:128]
        w_z = cb[0:EA, 128:256]
        whh_r = cb[:, 256:384]
        whh_z = cb[:, 384:512]
        whh_n = cb[:, 512:640]
        woutT = cb[:, 640:768]
        bhn = cb[:, 768:769]
        ones1 = cb[0:1, 769:897]
        bout1 = cb[0:1, 897:1025]

        # initial hidden state = 0
        h = []
        for c in range(n_chains):
            h0 = hpool.tile([H, bc], F16, name=f"h0_{c}", tag=f"h{c}")
            nc.gpsimd.memset(h0[:], 0.0)
            h.append(h0)

        # dummy sigmoid on the zeroed h0 pulls the ACT table load
        # (~2.7us) into the prologue, hidden behind the input DMAs
        warm = gpool.tile([H, 8], F16, name="warm_sb", tag="warm")
        nc.scalar.activation(warm[:], h[0][:, 0:8], AF.Sigmoid)

        # stream prefetch, 2 steps ahead
        streams = {}

        def issue_stream(t):
            st = stpool.tile([H, 2 * bl], F16, name="st_t", tag="st")
            dma = nc.sync.dma_start(st[:], st_d[t % n_steps])
            streams[t] = (st, dma)

        issue_stream(0)
        total_steps = n_steps * repeats
        if total_steps > 1:
            issue_stream(1)

        for t in range(total_steps):
            if t + 2 < total_steps:
                issue_stream(t + 2)
            st, st_dma = streams.pop(t)
            et = st[0:EA, 0:bl]
            xn = st[:, bl:2 * bl]

            for c in range(n_chains):
                ecol = et[:, c * bc:(c + 1) * bc]
                xcol = xn[:, c * bc:(c + 1) * bc]

                if split_rz:
                    r_ps = psrz.tile([H, bc], F32, name="r_ps", tag="rz")
                    z_ps = psz2.tile([H, bc], F32, name="z_ps", tag="z2")
                else:
                    rz_ps = psrz.tile([H, 2 * bc], F32, name="rz_ps", tag="rz")
                    r_ps = rz_ps[:, 0:bc]
                    z_ps = rz_ps[:, bc:2 * bc]
                hn_ps = pshn.tile([H, bc], F32, name="hn_ps", tag="hn")

                # pre-activations: input half first (no dependence on h),
                # then recurrent half accumulates on top
                nc.tensor.matmul(r_ps[:], w_r, ecol,
                                 start=True, stop=False)
                nc.tensor.matmul(z_ps[:], w_z, ecol,
                                 start=True, stop=False)
                nc.tensor.matmul(r_ps[:], whh_r, h[c][:],
                                 start=False, stop=True)
                nc.tensor.matmul(z_ps[:], whh_z, h[c][:],
                                 start=False, stop=True)
                nc.tensor.matmul(hn_ps[:], whh_n, h[c][:],
                                 start=True, stop=True)

                # Critical path per step is h -> hg_r MM -> sigmoid(r) ->
                # tt -> u -> tanh -> nzc -> h_new.  Everything z-related is
                # off-path: sigmoid(z) feeds zc = 1-z and zh = z*h, both on
                # GPSIMD, so only two DVE ops follow the tanh.
                prio = (lambda: tc.high_priority(offset=path_prio)) \
                    if path_prio else contextlib.nullcontext
                rz = gpool.tile([H, 2 * bc], F16, name="rz_sb", tag="rz_sb")
                tt = gpool.tile([H, bc], F16, name="tt_sb", tag="tt")
                u = gpool.tile([H, bc], F16, name="u_sb", tag="u")
                n_sb = gpool.tile([H, bc], F16, name="n_sb", tag="n")
                with prio():
                    if merged_sigma:
                        nc.scalar.activation(rz[:], rz_ps[:], AF.Sigmoid)
                    else:
                        nc.scalar.activation(rz[:, 0:bc], r_ps[:],
                                             AF.Sigmoid)
                        if sigma_z_early:
                            nc.scalar.activation(rz[:, bc:2 * bc], z_ps[:],
                                                 AF.Sigmoid)
                    # tt = (hn + b_hh_n) * r
                    spc = stt_pool_cols
                    dvc = bc - spc
                    if spc > 0:
                        nc.gpsimd.scalar_tensor_tensor(
                            tt[:, dvc:bc], hn_ps[:, dvc:bc], bhn,
                            rz[:, dvc:bc], op0=OP.add, op1=OP.mult)
                    if stt_split and spc == 0:
                        hb = bc // 2
                        nc.vector.scalar_tensor_tensor(
                            tt[:, 0:hb], hn_ps[:, 0:hb], bhn, rz[:, 0:hb],
                            op0=OP.add, op1=OP.mult)
                        nc.vector.scalar_tensor_tensor(
                            tt[:, hb:bc], hn_ps[:, hb:bc], bhn,
                            rz[:, hb:bc], op0=OP.add, op1=OP.mult)
                    elif dvc > 0:
                        nc.vector.scalar_tensor_tensor(
                            tt[:, 0:dvc], hn_ps[:, 0:dvc], bhn, rz[:, 0:dvc],
                            op0=OP.add, op1=OP.mult)
                    if u_alt and c == 1:
                        nc.gpsimd.tensor_add(u[:], tt[:], xcol)
                    else:
                        nc.vector.tensor_add(u[:], tt[:], xcol)
                    nc.scalar.activation(n_sb[:], u[:], AF.Tanh)

                if not merged_sigma and not sigma_z_early:
                    nc.scalar.activation(rz[:, bc:2 * bc], z_ps[:],
                                         AF.Sigmoid)
                zc = gpool.tile([H, bc], F16, name="zc_sb", tag="zc")
                if zc_dve_all or (zc_alt and c == 0):
                    nc.vector.tensor_scalar(zc[:], rz[:, bc:2 * bc], -1.0, 1.0,
                                            OP.mult, OP.add)
                else:
                    nc.gpsimd.tensor_scalar(zc[:], rz[:, bc:2 * bc], -1.0, 1.0,
                                            OP.mult, OP.add)
                zh = gpool.tile([H, bc], F16, name="zh_sb", tag="zh")
                if zh_on_dve is True or (zh_on_dve == "alt" and c == 1) \
                        or (zh_on_dve == "alt0" and c == 0):
                    nc.vector.tensor_mul(zh[:], rz[:, bc:2 * bc], h[c][:])
                else:
                    nc.gpsimd.tensor_mul(zh[:], rz[:, bc:2 * bc], h[c][:])

                # h_new = n*(1-z) + z*h
                nzc = gpool.tile([H, bc], F16, name="nzc_sb", tag="nzc")
                h_new = hpool.tile([H, bc], F16, name=f"hn_{c}", tag=f"h{c}")
                with prio():
                    if nzc_alt and c == 0:
                        nc.gpsimd.tensor_mul(nzc[:], n_sb[:], zc[:])
                    else:
                        nc.vector.tensor_mul(nzc[:], n_sb[:], zc[:])
                    nc.vector.tensor_add(h_new[:], nzc[:], zh[:])
                h[c] = h_new

        # output head: out[b, :] = h_last[:, b] . W_outT + b_out
        for c in range(n_chains):
            for bt in range(bc // H):
                o_ps = pshn.tile([H, OUT], F32, name="o_ps", tag="hn")
                lhs = h[c][:, bt * H:(bt + 1) * H]
                nc.tensor.matmul(o_ps[:], lhs, woutT,
                                 start=True, stop=False)
                nc.tensor.matmul(o_ps[:], ones1, bout1,
                                 start=False, stop=True)
                o_sb = gpool.tile([H, OUT], F32, name="o_sb", tag="osb")
                nc.scalar.activation(o_sb[:], o_ps[:], AF.Copy)
                r0 = c * bc + bt * H
                nc.sync.dma_start(out_d[r0:r0 + H], o_sb[:])

        pools = [pshn] + ([psz2] if split_rz else []) + [psrz, gpool, hpool, stpool, cpool]
        for p in pools:
            p.release()

    nc.compile()
    return nc


def _host_prep_v1(inputs, n_steps=L, bl=BL):
    """Shared (weights) + per-core (streams) host-side layout prep."""
    x = np.asarray(inputs["x"]).astype(np.int64)
    embed = np.asarray(inputs["embed"], dtype=np.float32)
    W_ih = np.asarray(inputs["W_ih"], dtype=np.float32)
    W_hh = np.asarray(inputs["W_hh"], dtype=np.float32)
    b_ih = np.asarray(inputs["b_ih"], dtype=np.float32)
    b_hh = np.asarray(inputs["b_hh"], dtype=np.float32)
    W_out = np.asarray(inputs["W_out"], dtype=np.float32)
    b_out = np.asarray(inputs["b_out"], dtype=np.float32)

    def aug_w(g):
        # lhsT [EA, H]: rows 0:64 = W_ih[g].T, row 64 = combined bias, rest 0
        w = np.zeros((EA, H), np.float16)
        w[:E] = W_ih[g * H:(g + 1) * H].T.astype(np.float16)
        w[E] = (b_ih[g * H:(g + 1) * H] + b_hh[g * H:(g + 1) * H]).astype(np.float16)
        return w

    cb = np.zeros((H, NCONST), np.float16)
    cb[0:EA, 0:128] = aug_w(0)
    cb[0:EA, 128:256] = aug_w(1)
    cb[:, 256:384] = W_hh[0:H].T.astype(np.float16)
    cb[:, 384:512] = W_hh[H:2 * H].T.astype(np.float16)
    cb[:, 512:640] = W_hh[2 * H:3 * H].T.astype(np.float16)
    cb[:, 640:768] = W_out.T.astype(np.float16)
    cb[:, 768] = b_hh[2 * H:3 * H].astype(np.float16)
    cb[0, 769:897] = 1.0
    cb[0, 897:1025] = b_out.astype(np.float16)

    # embed table with ones column for the bias rows of the aug weights
    T_aug = np.zeros((NV, EA), np.float16)
    T_aug[:, :E] = embed.astype(np.float16)
    T_aug[:, E] = 1.0
    # n-gate input projection table (bias folded in)
    G_n = (embed @ W_ih[2 * H:3 * H].T + b_ih[2 * H:3 * H]).astype(np.float16)

    per_core = []
    n_cores = x.shape[0] // bl
    for i in range(n_cores):
        xc = x[i * bl:(i + 1) * bl, :n_steps]              # [bl, n_steps]
        stream = np.zeros((n_steps, H, 2 * bl), np.float16)
        stream[:, 0:EA, 0:bl] = T_aug[xc].transpose(1, 2, 0)
        stream[:, :, bl:2 * bl] = G_n[xc].transpose(1, 2, 0)
        per_core.append({"stream": stream, "consts": cb})
    return per_core


import os

VARIANT = os.environ.get("BASS_VARIANT", "v3")


def build_nc(**kw):
    return build_v1(**kw) if VARIANT == "v1" else build_v3(**kw)


def _host_prep(inputs, n_steps=L, bl=BL):
    if VARIANT == "v1":
        return _host_prep_v1(inputs, n_steps=n_steps, bl=bl)
    return _host_prep_v3(inputs, n_steps=n_steps, bl=bl)


def _run(inputs, trace=False, **kw):
    key = "full_" + VARIANT
    if key not in _BUILD_CACHE:
        _BUILD_CACHE[key] = build_nc()
    nc = _BUILD_CACHE[key]
    in_maps = _host_prep(inputs)
    res = run_bass_kernel_spmd(nc, in_maps, list(range(N_CORES)), trace=trace, **kw)
    out = np.concatenate([res.results[i]["out"] for i in range(N_CORES)], axis=0)
    return out.astype(np.float32), res


def kernel(**inputs) -> np.ndarray:
    out, _ = _run(inputs)
    return out



# revision 28
# speedup vs baseline: 1.4776x; 1.1465x over previous
"""GRU sequence encoder (DiscSeqRNNEncoder) for 8x TRN2 NeuronCores.

Strategy: pure data-parallel over the batch (1024 rows/core).  On-device
everything lives in "transposed" layout [hidden/gate on partitions, batch on
free] so the recurrent state never needs a transpose.  Host-side prep does
the embedding gather into a transposed fp16 stream with an appended ones-row
(so the PE matmuls fold all biases in), plus the n-gate input projection
table gather.  Per time step the PE accumulates r/z pre-activations
(input + recurrent halves) directly in PSUM, ScalarE applies sigmoids and a
tanh, and DVE/GPSIMD do the remaining elementwise ops (fp16, with a fused
scalar_tensor_tensor for r*(hn+b_hh_n)).  Two half-batch chains (512 each)
pipeline through the engines to hide the serial dependency of the
recurrence.

All constants arrive in ONE packed DMA and each step's inputs (embeddings
stream + n-gate input projections) in ONE DMA, prefetched two steps ahead.
Multi-wait legalization (one sync wait per hardware instruction) is handled
by Bacc.compile()'s generate_event_semaphores pass.
"""

import numpy as np

import concourse.bass as bass
import concourse.tile as tile
from concourse import bacc
from concourse import mybir
from concourse.bass_utils import run_bass_kernel_spmd

F16 = mybir.dt.float16
F32 = mybir.dt.float32

B, L = 8192, 64
NV, E, H, OUT = 1000, 64, 128, 128
N_CORES = 8
BL = B // N_CORES          # batch rows per core
EA = 80                    # embed (64) + ones row (1) padded to 80 partitions
NCONST = 1025              # packed const block free size
NCONST3 = 2561             # v3 consts (bhn row, ones_bl row, negated U blocks)

_BUILD_CACHE = {}


def build_v3(n_steps=L, bl=BL, n_chains=2, prefetch=4, repeats=1,
             gates_bufs=12, h_bufs=3,
             tt_eng="vv", u_eng="vv", d_eng="vv", dl_eng="vv", hup_eng="vp",
             sz_late=False, mm_split=1, prio=None, dl_split=1, sz_dep=False,
             sz_split=1, zh_split=False, zh_eng="vv", hm_eng="pp",
             hnew_eng="vp", tt_csplit=0, u_csplit=0):
    """Delta-form GRU: persistent PSUM accumulators bank_r/z/n hold the
    running recurrent pre-activations (U·h_t + bias); each step adds
    W·Δe_t (embedding delta, streamed) and U·Δ_{t-1} (Δ = h-increment).
    z-gate weights are negated on host so σ yields z' = 1-z directly:
        n  = tanh(xn + r·bank_n)         bank_n = U_n·h + b_hh_n
        Δ  = z'·(n − h);   h += Δ
    Engine flags: per-op 2-char string, one of 'v' (DVE) / 'p' (GPSIMD)
    per chain.
    """
    bc = bl // n_chains
    nc = bacc.Bacc("TRN2", target_bir_lowering=False, debug=False)

    st_d = nc.dram_tensor("stream", [n_steps, H, 2 * bl], F16,
                          kind="ExternalInput").ap()
    cb_d = nc.dram_tensor("consts", [H, NCONST3], F16, kind="ExternalInput").ap()
    out_d = nc.dram_tensor("out", [bl, OUT], F32, kind="ExternalOutput").ap()

    AF = mybir.ActivationFunctionType
    OP = mybir.AluOpType
    import contextlib

    def eng(flag, c):
        flag = (flag * n_chains)[:n_chains] if len(flag) < n_chains else flag
        return nc.vector if flag[c] == "v" else nc.gpsimd

    with tile.TileContext(nc) as tc:
        cpool = tc.alloc_tile_pool(name="consts", bufs=1)
        stpool = tc.alloc_tile_pool(name="stream", bufs=prefetch)
        hpool = tc.alloc_tile_pool(name="hstate", bufs=h_bufs)
        gpool = tc.alloc_tile_pool(name="gates", bufs=gates_bufs)
        bpool = tc.alloc_tile_pool(name="banks", bufs=1, space="PSUM")

        cb = cpool.tile([H, NCONST3], F16, name="cb_sb")
        nc.sync.dma_start(cb[:], cb_d[:])
        w_r = cb[0:EA, 0:128]
        w_z = cb[0:EA, 128:256]          # negated on host
        whh_r = cb[:, 256:384]
        whh_z = cb[:, 384:512]           # negated on host
        whh_n = cb[:, 512:640]
        woutT = cb[:, 640:768]
        ones1 = cb[0:1, 769:897]
        bout1 = cb[0:1, 897:1025]
        bhn_row = cb[0:1, 1025:1153]     # b_hh_n as a [1,128] lhsT row
        ones_bl = cb[0:1, 1153:1153 + bl]  # ones row for rank-1 bias fills
        whh_rm = cb[:, 2177:2305]        # -W_hh_r.T (zh_split)
        whh_zm = cb[:, 2305:2433]        # +W_hh_z.T (= negated z'-weights)
        whh_nm = cb[:, 2433:2561]        # -W_hh_n.T

        # persistent PSUM accumulators; pack several chains' same-gate
        # accumulators into one 2KB bank when bc is small enough
        grp = max(1, 2048 // (bc * 4))

        def alloc_banks(prefix):
            group_tiles = []
            for g in range((n_chains + grp - 1) // grp):
                w = min(grp, n_chains - g * grp)
                group_tiles.append(
                    bpool.tile([H, w * bc], F32,
                               name=f"{prefix}g{g}", tag=f"{prefix}g{g}"))
            views = []
            for c in range(n_chains):
                g, o = divmod(c, grp)
                views.append(group_tiles[g][:, o * bc:(o + 1) * bc])
            return views

        bank_r = alloc_banks("bkr")
        bank_z = alloc_banks("bkz")
        bank_n = alloc_banks("bkn")

        h = []
        delta = [None] * n_chains
        zh_tiles = [None] * n_chains
        for c in range(n_chains):
            h0 = hpool.tile([H, bc], F16, name=f"h0_{c}", tag=f"h{c}")
            nc.gpsimd.memset(h0[:], 0.0)
            h.append(h0)

        # dummy sigmoid pulls the ACT table load into the prologue
        warm = gpool.tile([H, 8], F16, name="warm_sb", tag="warm")
        nc.scalar.activation(warm[:], h[0][:, 0:8], AF.Sigmoid)
        # dummy matmuls burn through the PE p-state ramp while DMAs land
        # (they write into bank_r[0], which the first real MM resets with
        # start=True, so the garbage is harmless)
        for _ in range(8):
            nc.tensor.matmul(bank_r[0][:], h[0][:, 0:H], h[0][:],
                             start=True, stop=True, skip_group_check=True)

        streams = {}

        def issue_stream(t):
            st = stpool.tile([H, 2 * bl], F16, name="st_t", tag="st")
            dma = nc.sync.dma_start(st[:], st_d[t % n_steps])
            streams[t] = (st, dma)

        issue_stream(0)
        total_steps = n_steps * repeats
        if total_steps > 1:
            issue_stream(1)

        def input_mms(t, c, st):
            # W·Δe_t accumulation — depends only on the streamed Δe, so it
            # can run as soon as the banks' previous-step reads are done.
            de = st[0:EA, 0:bl]
            decol = de[:, c * bc:(c + 1) * bc]
            first = t == 0
            nc.tensor.matmul(bank_r[c][:], w_r, decol,
                             start=first, stop=first,
                             skip_group_check=True)
            nc.tensor.matmul(bank_z[c][:], w_z, decol,
                             start=first, stop=first,
                             skip_group_check=True)
            if first:
                # ones row → rank-1 bias fill of bank_n
                nc.tensor.matmul(bank_n[c][:], bhn_row,
                                 ones_bl[:, c * bc:(c + 1) * bc],
                                 start=True, stop=True,
                                 skip_group_check=True)

        for c in range(n_chains):
            input_mms(0, c, streams[0][0])

        for t in range(total_steps):
            if t + 2 < total_steps:
                issue_stream(t + 2)
            st, _ = streams.pop(t)
            xn = st[:, bl:2 * bl]

            for c in range(n_chains):
                xcol = xn[:, c * bc:(c + 1) * bc]

                if t > 0:
                    dl = delta[c]
                    for s in range(mm_split):
                        sl = slice(s * bc // mm_split,
                                   (s + 1) * bc // mm_split)
                        nc.tensor.matmul(bank_r[c][:, sl], whh_r, dl[:, sl],
                                         start=False, stop=True,
                                         skip_group_check=True)
                    nc.tensor.matmul(bank_n[c][:], whh_n, dl[:],
                                     start=False, stop=True,
                                     skip_group_check=True)
                    for s in range(mm_split):
                        sl = slice(s * bc // mm_split,
                                   (s + 1) * bc // mm_split)
                        nc.tensor.matmul(bank_z[c][:, sl], whh_z, dl[:, sl],
                                         start=False, stop=True,
                                         skip_group_check=True)
                    if zh_split:
                        # subtractive half: U_neg · zh_{t-1} (zh was ready
                        # mid-previous-step; these run off the critical path)
                        zh_prev = zh_tiles[c]
                        nc.tensor.matmul(bank_r[c][:], whh_rm, zh_prev[:],
                                         start=False, stop=True,
                                         skip_group_check=True)
                        nc.tensor.matmul(bank_n[c][:], whh_nm, zh_prev[:],
                                         start=False, stop=True,
                                         skip_group_check=True)
                        nc.tensor.matmul(bank_z[c][:], whh_zm, zh_prev[:],
                                         start=False, stop=True,
                                         skip_group_check=True)

                pctx = (lambda: tc.high_priority(offset=prio)) \
                    if prio else contextlib.nullcontext
                rz = gpool.tile([H, 2 * bc], F16, name="rz_sb", tag=f"rz{c}")
                tt = gpool.tile([H, bc], F16, name="tt_sb", tag=f"tt{c}")
                u = gpool.tile([H, bc], F16, name="u_sb", tag=f"u{c}")
                n_sb = gpool.tile([H, bc], F16, name="n_sb", tag=f"n{c}")
                with pctx():
                    nc.scalar.activation(rz[:, 0:bc], bank_r[c][:], AF.Sigmoid)
                if not sz_late and not sz_dep:
                    for s in range(sz_split):
                        sl = slice(s * bc // sz_split, (s + 1) * bc // sz_split)
                        nc.scalar.activation(rz[:, bc + sl.start:bc + sl.stop],
                                             bank_z[c][:, sl], AF.Sigmoid)
                with pctx():
                    if tt_csplit:
                        sp = tt_csplit
                        nc.vector.tensor_mul(tt[:, 0:sp], bank_n[c][:, 0:sp],
                                             rz[:, 0:sp])
                        nc.gpsimd.tensor_mul(tt[:, sp:bc], bank_n[c][:, sp:bc],
                                             rz[:, sp:bc])
                    else:
                        eng(tt_eng, c).tensor_mul(tt[:], bank_n[c][:],
                                                  rz[:, 0:bc])
                    if u_csplit:
                        sp = u_csplit
                        nc.vector.tensor_add(u[:, 0:sp], tt[:, 0:sp],
                                             xcol[:, 0:sp])
                        nc.gpsimd.tensor_add(u[:, sp:bc], tt[:, sp:bc],
                                             xcol[:, sp:bc])
                    else:
                        eng(u_eng, c).tensor_add(u[:], tt[:], xcol)
                    nc.scalar.activation(n_sb[:], u[:], AF.Tanh)
                if sz_dep:
                    # zero bias derived from the tanh output: forces the
                    # in-order ACT queue to schedule this sigma_z strictly
                    # after the same chain's tanh (it would otherwise
                    # greedily run first and head-of-line block the tanh)
                    bz = gpool.tile([H, 1], F32, name="bz_sb", tag=f"bz{c}")
                    nc.vector.tensor_scalar(bz[:], n_sb[:, 0:1], 0.0, 0.0,
                                            OP.mult, OP.mult)
                    nc.scalar.activation(rz[:, bc:2 * bc], bank_z[c][:],
                                         AF.Sigmoid, bias=bz[:])
                elif sz_late:
                    nc.scalar.activation(rz[:, bc:2 * bc], bank_z[c][:],
                                         AF.Sigmoid)

                if t + 1 < total_steps:
                    input_mms(t + 1, c, streams[t + 1][0])

                h_new = hpool.tile([H, bc], F16, name=f"hn_{c}", tag=f"h{c}")
                if zh_split:
                    # zh = z'*h (early, off-path); hm = h - zh (off-path);
                    # zn = z'*n (path -> feeds next step's U MMs); h' = hm+zn
                    zh_t = gpool.tile([H, bc], F16, name="zh_sb", tag=f"zh{c}")
                    hm = gpool.tile([H, bc], F16, name="hm_sb", tag=f"hm{c}")
                    zn = gpool.tile([H, bc], F16, name="zn_sb", tag=f"zn{c}")
                    eng(zh_eng, c).tensor_mul(zh_t[:], rz[:, bc:2 * bc],
                                              h[c][:])
                    eng(hm_eng, c).tensor_sub(hm[:], h[c][:], zh_t[:])
                    with pctx():
                        eng(d_eng, c).tensor_mul(zn[:], rz[:, bc:2 * bc],
                                                 n_sb[:])
                    eng(hnew_eng, c).tensor_add(h_new[:], hm[:], zn[:])
                    zh_tiles[c] = zh_t
                    delta[c] = zn
                else:
                    d = gpool.tile([H, bc], F16, name="d_sb", tag=f"d{c}")
                    dl_new = gpool.tile([H, bc], F16, name="dl_sb", tag=f"dl{c}")
                    with pctx():
                        eng(d_eng, c).tensor_sub(d[:], n_sb[:], h[c][:])
                        for s in range(dl_split):
                            sl = slice(s * bc // dl_split,
                                       (s + 1) * bc // dl_split)
                            eng(dl_eng, c).tensor_mul(
                                dl_new[:, sl],
                                rz[:, bc + sl.start:bc + sl.stop],
                                d[:, sl])
                    eng(hup_eng, c).tensor_add(h_new[:], h[c][:], dl_new[:])
                    delta[c] = dl_new
                h[c] = h_new

        # output head: out[b, :] = h_last[:, b] . W_outT + b_out
        # PSUM result -> one packed SBUF tile -> ONE output DMA
        opool = tc.alloc_tile_pool(name="outps", bufs=2, space="PSUM")
        n_blk = bl // H
        oapool = tc.alloc_tile_pool(name="oall", bufs=1)
        o_all = oapool.tile([H, n_blk * OUT], F32, name="o_all", tag="oall")
        for c in range(n_chains):
            for bt in range(bc // H):
                o_ps = opool.tile([H, OUT], F32, name="o_ps", tag="ops")
                lhs = h[c][:, bt * H:(bt + 1) * H]
                nc.tensor.matmul(o_ps[:], lhs, woutT,
                                 start=True, stop=False,
                                 skip_group_check=True)
                nc.tensor.matmul(o_ps[:], ones1, bout1,
                                 start=False, stop=True,
                                 skip_group_check=True)
                blk = c * (bc // H) + bt
                nc.vector.tensor_copy(o_all[:, blk * OUT:(blk + 1) * OUT],
                                      o_ps[:])
        nc.sync.dma_start(
            out_d.rearrange("(blk p) o -> p blk o", p=H), o_all[:])

        for p in [oapool, opool, bpool, gpool, hpool, stpool, cpool]:
            p.release()

    nc.compile()
    return nc


def _host_prep_v3(inputs, n_steps=L, bl=BL):
    """v3 layout: stream carries Δe (embedding deltas) + xn; z-gate
    weights negated; consts gain a b_hh_n lhsT row."""
    x = np.asarray(inputs["x"]).astype(np.int64)
    embed = np.asarray(inputs["embed"], dtype=np.float32)
    W_ih = np.asarray(inputs["W_ih"], dtype=np.float32)
    W_hh = np.asarray(inputs["W_hh"], dtype=np.float32)
    b_ih = np.asarray(inputs["b_ih"], dtype=np.float32)
    b_hh = np.asarray(inputs["b_hh"], dtype=np.float32)
    W_out = np.asarray(inputs["W_out"], dtype=np.float32)
    b_out = np.asarray(inputs["b_out"], dtype=np.float32)

    def aug_w(g, sign=1.0):
        w = np.zeros((EA, H), np.float16)
        w[:E] = (sign * W_ih[g * H:(g + 1) * H].T).astype(np.float16)
        w[E] = (sign * (b_ih[g * H:(g + 1) * H]
                        + b_hh[g * H:(g + 1) * H])).astype(np.float16)
        return w

    cb = np.zeros((H, NCONST3), np.float16)
    cb[0:EA, 0:128] = aug_w(0)
    cb[0:EA, 128:256] = aug_w(1, sign=-1.0)
    cb[:, 256:384] = W_hh[0:H].T.astype(np.float16)
    cb[:, 384:512] = (-W_hh[H:2 * H].T).astype(np.float16)
    cb[:, 512:640] = W_hh[2 * H:3 * H].T.astype(np.float16)
    cb[:, 640:768] = W_out.T.astype(np.float16)
    cb[0, 769:897] = 1.0
    cb[0, 897:1025] = b_out.astype(np.float16)
    cb[0, 1025:1153] = b_hh[2 * H:3 * H].astype(np.float16)
    cb[0, 1153:1153 + bl] = 1.0
    cb[:, 2177:2305] = (-W_hh[0:H].T).astype(np.float16)
    cb[:, 2305:2433] = W_hh[H:2 * H].T.astype(np.float16)
    cb[:, 2433:2561] = (-W_hh[2 * H:3 * H].T).astype(np.float16)

    T_aug = np.zeros((NV, EA), np.float32)
    T_aug[:, :E] = embed
    T_aug[:, E] = 1.0
    G_n = (embed @ W_ih[2 * H:3 * H].T + b_ih[2 * H:3 * H]).astype(np.float16)

    per_core = []
    n_cores = x.shape[0] // bl
    for i in range(n_cores):
        xc = x[i * bl:(i + 1) * bl, :n_steps]              # [bl, n_steps]
        e_seq = T_aug[xc]                                  # [bl, T, EA] f32
        de_seq = e_seq.copy()
        de_seq[:, 1:] -= e_seq[:, :-1]
        stream = np.zeros((n_steps, H, 2 * bl), np.float16)
        stream[:, 0:EA, 0:bl] = de_seq.astype(np.float16).transpose(1, 2, 0)
        stream[:, :, bl:2 * bl] = G_n[xc].transpose(1, 2, 0)
        per_core.append({"stream": stream, "consts": cb})
    return per_core


def build_v1(n_steps=L, bl=BL, n_chains=2, prefetch=4, repeats=1,
             psrz_bufs=3, pshn_bufs=2, gates_bufs=8, h_bufs=2,
             merged_sigma=False, zh_on_dve="alt", zc_alt=False, nzc_alt=False,
             path_prio=None, stt_split=False, split_rz=True, u_alt=False,
             stt_pool_cols=0, zc_dve_all=False, sigma_z_early=False):
    """Build the single-core Bass/Tile program (SPMD across 8 cores).

    repeats > 1 re-runs the whole recurrence (for differential wall-clock
    timing); numerics then chain h across repeats, which is fine for timing.
    """
    bc = bl // n_chains  # batch per chain
    nc = bacc.Bacc("TRN2", target_bir_lowering=False, debug=False)

    st_d = nc.dram_tensor("stream", [n_steps, H, 2 * bl], F16,
                          kind="ExternalInput").ap()
    cb_d = nc.dram_tensor("consts", [H, NCONST], F16, kind="ExternalInput").ap()
    out_d = nc.dram_tensor("out", [bl, OUT], F32, kind="ExternalOutput").ap()

    AF = mybir.ActivationFunctionType
    OP = mybir.AluOpType
    import contextlib

    with tile.TileContext(nc) as tc:
        cpool = tc.alloc_tile_pool(name="consts", bufs=1)
        stpool = tc.alloc_tile_pool(name="stream", bufs=prefetch)
        hpool = tc.alloc_tile_pool(name="hstate", bufs=h_bufs)
        gpool = tc.alloc_tile_pool(name="gates", bufs=gates_bufs)
        psrz = tc.alloc_tile_pool(name="psrz", bufs=psrz_bufs, space="PSUM")
        psz2 = (tc.alloc_tile_pool(name="psz2", bufs=psrz_bufs, space="PSUM")
                if split_rz else None)
        pshn = tc.alloc_tile_pool(name="pshn", bufs=pshn_bufs, space="PSUM")

        cb = cpool.tile([H, NCONST], F16, name="cb_sb")
        nc.sync.dma_start(cb[:], cb_d[:])
        w_r = cb[0:EA, 0:128]
        w_z = cb[0:EA, 128:256]
        whh_r = cb[:, 256:384]
        whh_z = cb[:, 384:512]
        whh_n = cb[:, 512:640]
        woutT = cb[:, 640:768]
        bhn = cb[:, 768:769]
        ones1 = cb[0:1, 769:897]
        bout1 = cb[0:1, 897:1025]

        # initial hidden state = 0
        h = []
        for c in range(n_chains):
            h0 = hpool.tile([H, bc], F16, name=f"h0_{c}", tag=f"h{c}")
            nc.gpsimd.memset(h0[:], 0.0)
            h.append(h0)

        # dummy sigmoid on the zeroed h0 pulls the ACT table load
        # (~2.7us) into the prologue, hidden behind the input DMAs
        warm = gpool.tile([H, 8], F16, name="warm_sb", tag="warm")
        nc.scalar.activation(warm[:], h[0][:, 0:8], AF.Sigmoid)

        # stream prefetch, 2 steps ahead
        streams = {}

        def issue_stream(t):
            st = stpool.tile([H, 2 * bl], F16, name="st_t", tag="st")
            dma = nc.sync.dma_start(st[:], st_d[t % n_steps])
            streams[t] = (st, dma)

        issue_stream(0)
        total_steps = n_steps * repeats
        if total_steps > 1:
            issue_stream(1)

        for t in range(total_steps):
            if t + 2 < total_steps:
                issue_stream(t + 2)
            st, st_dma = streams.pop(t)
            et = st[0:EA, 0:bl]
            xn = st[:, bl:2 * bl]

            for c in range(n_chains):
                ecol = et[:, c * bc:(c + 1) * bc]
                xcol = xn[:, c * bc:(c + 1) * bc]

                if split_rz:
                    r_ps = psrz.tile([H, bc], F32, name="r_ps", tag="rz")
                    z_ps = psz2.tile([H, bc], F32, name="z_ps", tag="z2")
                else:
                    rz_ps = psrz.tile([H, 2 * bc], F32, name="rz_ps", tag="rz")
                    r_ps = rz_ps[:, 0:bc]
                    z_ps = rz_ps[:, bc:2 * bc]
                hn_ps = pshn.tile([H, bc], F32, name="hn_ps", tag="hn")

                # pre-activations: input half first (no dependence on h),
                # then recurrent half accumulates on top
                nc.tensor.matmul(r_ps[:], w_r, ecol,
                                 start=True, stop=False)
                nc.tensor.matmul(z_ps[:], w_z, ecol,
                                 start=True, stop=False)
                nc.tensor.matmul(r_ps[:], whh_r, h[c][:],
                                 start=False, stop=True)
                nc.tensor.matmul(z_ps[:], whh_z, h[c][:],
                                 start=False, stop=True)
                nc.tensor.matmul(hn_ps[:], whh_n, h[c][:],
                                 start=True, stop=True)

                # Critical path per step is h -> hg_r MM -> sigmoid(r) ->
                # tt -> u -> tanh -> nzc -> h_new.  Everything z-related is
                # off-path: sigmoid(z) feeds zc = 1-z and zh = z*h, both on
                # GPSIMD, so only two DVE ops follow the tanh.
                prio = (lambda: tc.high_priority(offset=path_prio)) \
                    if path_prio else contextlib.nullcontext
                rz = gpool.tile([H, 2 * bc], F16, name="rz_sb", tag="rz_sb")
                tt = gpool.tile([H, bc], F16, name="tt_sb", tag="tt")
                u = gpool.tile([H, bc], F16, name="u_sb", tag="u")
                n_sb = gpool.tile([H, bc], F16, name="n_sb", tag="n")
                with prio():
                    if merged_sigma:
                        nc.scalar.activation(rz[:], rz_ps[:], AF.Sigmoid)
                    else:
                        nc.scalar.activation(rz[:, 0:bc], r_ps[:],
                                             AF.Sigmoid)
                        if sigma_z_early:
                            nc.scalar.activation(rz[:, bc:2 * bc], z_ps[:],
                                                 AF.Sigmoid)
                    # tt = (hn + b_hh_n) * r
                    spc = stt_pool_cols
                    dvc = bc - spc
                    if spc > 0:
                        nc.gpsimd.scalar_tensor_tensor(
                            tt[:, dvc:bc], hn_ps[:, dvc:bc], bhn,
                            rz[:, dvc:bc], op0=OP.add, op1=OP.mult)
                    if stt_split and spc == 0:
                        hb = bc // 2
                        nc.vector.scalar_tensor_tensor(
                            tt[:, 0:hb], hn_ps[:, 0:hb], bhn, rz[:, 0:hb],
                            op0=OP.add, op1=OP.mult)
                        nc.vector.scalar_tensor_tensor(
                            tt[:, hb:bc], hn_ps[:, hb:bc], bhn,
                            rz[:, hb:bc], op0=OP.add, op1=OP.mult)
                    elif dvc > 0:
                        nc.vector.scalar_tensor_tensor(
                            tt[:, 0:dvc], hn_ps[:, 0:dvc], bhn, rz[:, 0:dvc],
                            op0=OP.add, op1=OP.mult)
                    if u_alt and c == 1:
                        nc.gpsimd.tensor_add(u[:], tt[:], xcol)
                    else:
                        nc.vector.tensor_add(u[:], tt[:], xcol)
                    nc.scalar.activation(n_sb[:], u[:], AF.Tanh)

                if not merged_sigma and not sigma_z_early:
                    nc.scalar.activation(rz[:, bc:2 * bc], z_ps[:],
                                         AF.Sigmoid)
                zc = gpool.tile([H, bc], F16, name="zc_sb", tag="zc")
                if zc_dve_all or (zc_alt and c == 0):
                    nc.vector.tensor_scalar(zc[:], rz[:, bc:2 * bc], -1.0, 1.0,
                                            OP.mult, OP.add)
                else:
                    nc.gpsimd.tensor_scalar(zc[:], rz[:, bc:2 * bc], -1.0, 1.0,
                                            OP.mult, OP.add)
                zh = gpool.tile([H, bc], F16, name="zh_sb", tag="zh")
                if zh_on_dve is True or (zh_on_dve == "alt" and c == 1) \
                        or (zh_on_dve == "alt0" and c == 0):
                    nc.vector.tensor_mul(zh[:], rz[:, bc:2 * bc], h[c][:])
                else:
                    nc.gpsimd.tensor_mul(zh[:], rz[:, bc:2 * bc], h[c][:])

                # h_new = n*(1-z) + z*h
                nzc = gpool.tile([H, bc], F16, name="nzc_sb", tag="nzc")
                h_new = hpool.tile([H, bc], F16, name=f"hn_{c}", tag=f"h{c}")
                with prio():
                    if nzc_alt and c == 0:
                        nc.gpsimd.tensor_mul(nzc[:], n_sb[:], zc[:])
                    else:
                        nc.vector.tensor_mul(nzc[:], n_sb[:], zc[:])
                    nc.vector.tensor_add(h_new[:], nzc[:], zh[:])
                h[c] = h_new

        # output head: out[b, :] = h_last[:, b] . W_outT + b_out
        for c in range(n_chains):
            for bt in range(bc // H):
                o_ps = pshn.tile([H, OUT], F32, name="o_ps", tag="hn")
                lhs = h[c][:, bt * H:(bt + 1) * H]
                nc.tensor.matmul(o_ps[:], lhs, woutT,
                                 start=True, stop=False)
                nc.tensor.matmul(o_ps[:], ones1, bout1,
                                 start=False, stop=True)
                o_sb = gpool.tile([H, OUT], F32, name="o_sb", tag="osb")
                nc.scalar.activation(o_sb[:], o_ps[:], AF.Copy)
                r0 = c * bc + bt * H
                nc.sync.dma_start(out_d[r0:r0 + H], o_sb[:])

        pools = [pshn] + ([psz2] if split_rz else []) + [psrz, gpool, hpool, stpool, cpool]
        for p in pools:
            p.release()

    nc.compile()
    return nc


def _host_prep_v1(inputs, n_steps=L, bl=BL):
    """Shared (weights) + per-core (streams) host-side layout prep."""
    x = np.asarray(inputs["x"]).astype(np.int64)
    embed = np.asarray(inputs["embed"], dtype=np.float32)
    W_ih = np.asarray(inputs["W_ih"], dtype=np.float32)
    W_hh = np.asarray(inputs["W_hh"], dtype=np.float32)
    b_ih = np.asarray(inputs["b_ih"], dtype=np.float32)
    b_hh = np.asarray(inputs["b_hh"], dtype=np.float32)
    W_out = np.asarray(inputs["W_out"], dtype=np.float32)
    b_out = np.asarray(inputs["b_out"], dtype=np.float32)

    def aug_w(g):
        # lhsT [EA, H]: rows 0:64 = W_ih[g].T, row 64 = combined bias, rest 0
        w = np.zeros((EA, H), np.float16)
        w[:E] = W_ih[g * H:(g + 1) * H].T.astype(np.float16)
        w[E] = (b_ih[g * H:(g + 1) * H] + b_hh[g * H:(g + 1) * H]).astype(np.float16)
        return w

    cb = np.zeros((H, NCONST), np.float16)
    cb[0:EA, 0:128] = aug_w(0)
    cb[0:EA, 128:256] = aug_w(1)
    cb[:, 256:384] = W_hh[0:H].T.astype(np.float16)
    cb[:, 384:512] = W_hh[H:2 * H].T.astype(np.float16)
    cb[:, 512:640] = W_hh[2 * H:3 * H].T.astype(np.float16)
    cb[:, 640:768] = W_out.T.astype(np.float16)
    cb[:, 768] = b_hh[2 * H:3 * H].astype(np.float16)
    cb[0, 769:897] = 1.0
    cb[0, 897:1025] = b_out.astype(np.float16)

    # embed table with ones column for the bias rows of the aug weights
    T_aug = np.zeros((NV, EA), np.float16)
    T_aug[:, :E] = embed.astype(np.float16)
    T_aug[:, E] = 1.0
    # n-gate input projection table (bias folded in)
    G_n = (embed @ W_ih[2 * H:3 * H].T + b_ih[2 * H:3 * H]).astype(np.float16)

    per_core = []
    n_cores = x.shape[0] // bl
    for i in range(n_cores):
        xc = x[i * bl:(i + 1) * bl, :n_steps]              # [bl, n_steps]
        stream = np.zeros((n_steps, H, 2 * bl), np.float16)
        stream[:, 0:EA, 0:bl] = T_aug[xc].transpose(1, 2, 0)
        stream[:, :, bl:2 * bl] = G_n[xc].transpose(1, 2, 0)
        per_core.append({"stream": stream, "consts": cb})
    return per_core


import os

VARIANT = os.environ.get("BASS_VARIANT", "v3")


def build_nc(**kw):
    return build_v1(**kw) if VARIANT == "v1" else build_v3(**kw)


def _host_prep(inputs, n_steps=L, bl=BL):
    if VARIANT == "v1":
        return _host_prep_v1(inputs, n_steps=n_steps, bl=bl)
    return _host_prep_v3(inputs, n_steps=n_steps, bl=bl)


def _run(inputs, trace=False, **kw):
    key = "full_" + VARIANT
    if key not in _BUILD_CACHE:
        _BUILD_CACHE[key] = build_nc()
    nc = _BUILD_CACHE[key]
    in_maps = _host_prep(inputs)
    res = run_bass_kernel_spmd(nc, in_maps, list(range(N_CORES)), trace=trace, **kw)
    out = np.concatenate([res.results[i]["out"] for i in range(N_CORES)], axis=0)
    return out.astype(np.float32), res


def kernel(**inputs) -> np.ndarray:
    out, _ = _run(inputs)
    return out

